# revision 46
# baseline (speedup 1.0000x reference)
"""Trainium2 Bass kernel for nn_MDFO (CNL + PNL non-local blocks + CBAM + fusion).

Restructured v7 (pure data-parallel, B=8 over 8 cores, params replicated):
  - bf16 inputs (x, x0) uploaded from host; bf16 output, fp32 on host.
  - all constants packed into three blob DMAs (early-bf16, late-bf16, f32).
  - theta never applied per-pixel and att never materialized: stage A only
    accumulates G = x @ ph^T and colsum(ph); fold1 = G^T WB + cs (x) bb with
    WB = w_th @ [w_tyT|w_cnlW] folded host-side; fold2 = w_gT @ fold1.
  - Y2 never materialized: WS = S2T-halves @ w_pnlW lets z contract T2
    directly; S blocks emitted transposed with paired G columns.
  - z emission split across Act (ident-matmul + bias path) and DVE stt,
    with the CBAM mean accumulated via accum_out and the channel max via
    two rolling chains (first half's pixel reduce runs during z).
  - mean mapT built by tiny per-column matmuls (stationary = z 64-col
    block, moving = ca column); max map via Pool partition_all_reduce into
    replicated f32 rows + tiny f32 column transposes; Sdy has no perm.
  - sigmoid broadcast straight from sig2d via broadcast-stationary
    w*ident-column matmuls (no sigrow extraction).
  - (1-w)*x computed on Pool and pre-written to out_d during the idle DMA
    window; the final z*ca*sigb multiply is accumulated on top with
    software-DGE accum DMAs, eliminating the final add pass entirely.
"""
import sys

import numpy as np

sys.path.insert(0, "/opt/trn_rl_repo")

import ml_dtypes  # noqa: E402

import concourse.bass as bass  # noqa: E402
import concourse.bacc as bacc  # noqa: E402
import concourse.tile as tile  # noqa: E402
from concourse import mybir  # noqa: E402
from concourse.bass_utils import run_bass_kernel_spmd  # noqa: E402

EPS = 1e-5
F32 = mybir.dt.float32
F32R = mybir.dt.float32r
BF16 = mybir.dt.bfloat16
AF = mybir.ActivationFunctionType
ALU = mybir.AluOpType

Ch, Cl, H, W = 256, 128, 64, 64
N = H * W            # 4096
M = N // 2           # 2048
r = Cl // 2          # 64

# blob layouts: name -> (col offset, cols, rows)
CBA_COLS = 768   # early bf16 blob
CBA = {'w_x0cat': (0, 256, 128), 'b_x0cat': (256, 256, 128),
       'ones1': (512, 128, 1), 'ident_bf': (640, 128, 128)}
CBB_COLS = 2752  # late bf16 blob
CBB = {'w_th2': (0, 128, 128), 'w_pnlW': (128, 256, 128),
       'Kcat2': (384, 896, 64), 'Sdy7': (1280, 448, 64),
       'wident': (1728, 64, 64), 'WB0': (1792, 320, 128),
       'WB1': (2112, 320, 128), 'bb': (2432, 320, 1)}
CF_COLS = 422    # f32 blob
CF = {'w_gT': (0, 128, 128), 'b_g': (128, 2, 128),
      'b_th2': (130, 1, 64), 'b2': (131, 2, 128), 'fc1T': (133, 32, 128),
      'fc2T': (165, 256, 16), 'onef': (421, 1, 1)}


def _R(ap):
    return ap.bitcast(F32R)


def fold_params(inp):
    """Host-side constant folding into three blob arrays."""
    f = {}
    scale1 = inp['cnl_bn_g'] / np.sqrt(inp['cnl_bn_v'] + EPS)
    cnl_bf = (inp['cnl_W_b'] * scale1 + inp['cnl_bn_b']
              - inp['cnl_bn_m'] * scale1).astype(np.float32)
    scale2 = inp['pnl_bn_g'] / np.sqrt(inp['pnl_bn_v'] + EPS)
    pnl_bf = (inp['pnl_W_b'] * scale2 + inp['pnl_bn_b']
              - inp['pnl_bn_m'] * scale2).astype(np.float32)
    w_fuse = float(inp['fusion_weight'])
    f['w_fuse'] = w_fuse

    cbA = np.zeros((128, CBA_COLS), dtype=np.float32)
    cbB = np.zeros((128, CBB_COLS), dtype=np.float32)
    cf = np.zeros((128, CF_COLS), dtype=np.float32)

    def put(blob, table, name, arr):
        off, cols, rows = table[name]
        blob[:rows, off:off + cols] = arr

    put(cbA, CBA, 'w_x0cat', np.concatenate([
        inp['cnl_phi_w'].T, inp['pnl_phi_w'].T, (inp['pnl_g_w'] / M).T],
        axis=1))
    brow = np.concatenate([inp['cnl_phi_b'], inp['pnl_phi_b'],
                           inp['pnl_g_b'] / M])
    put(cbA, CBA, 'b_x0cat', np.tile(brow[None, :], (128, 1)))
    put(cbA, CBA, 'ones1', np.ones((1, 128), dtype=np.float32))
    put(cbA, CBA, 'ident_bf', np.eye(128, dtype=np.float32))

    th2 = inp['pnl_theta_w'].T
    put(cbB, CBB, 'w_th2', np.concatenate([th2[:128], th2[128:]], axis=1))
    w_pnlW = (scale2[:, None] * inp['pnl_W_w']).T
    put(cbB, CBB, 'w_pnlW', np.concatenate([w_pnlW, w_pnlW], axis=0))
    # sa conv banded mats; only 1/256 fold on the mean channel (no w folds)
    sa_w = np.asarray(inp['sa_conv_w'][0], dtype=np.float32).copy()
    sa_w[0] /= 256.0
    Kcat = np.zeros((2, 64, 7 * 64), dtype=np.float32)
    for ch in range(2):
        for dy in range(7):
            for dx in range(7):
                w_ = sa_w[ch, dy, dx]
                if w_ == 0.0:
                    continue
                for x in range(64):
                    xq = x + dx - 3
                    if 0 <= xq < 64:
                        Kcat[ch, xq, dy * 64 + x] = w_
    put(cbB, CBB, 'Kcat2', np.concatenate([Kcat[0], Kcat[1]], axis=1))
    Sdy = np.zeros((64, 7 * 64), dtype=np.float32)
    for dy in range(7):
        for y in range(64):
            yp = y + dy - 3
            if 0 <= yp < 64:
                Sdy[yp, dy * 64 + y] = 1.0
    put(cbB, CBB, 'Sdy7', Sdy)
    put(cbB, CBB, 'wident', w_fuse * np.eye(64, dtype=np.float32))
    # fold theta through the fold-1 weights: fold1 = G^T WB + cs (x) bb
    w_big = np.concatenate([
        (inp['pnl_theta_w'] @ (scale1[:, None] * inp['cnl_W_w'])).T,
        (scale1[:, None] * inp['cnl_W_w']).T], axis=1)
    WB = inp['cnl_theta_w'].T @ w_big
    put(cbB, CBB, 'WB0', WB[:128])
    put(cbB, CBB, 'WB1', WB[128:])
    put(cbB, CBB, 'bb', (inp['cnl_theta_b'] @ w_big)[None, :])

    put(cf, CF, 'w_gT', inp['cnl_g_w'] / Cl)
    bgc = (inp['cnl_g_b'] / Cl)[:, None]
    put(cf, CF, 'b_g', np.concatenate([bgc, bgc], axis=1))
    put(cf, CF, 'b_th2', (inp['pnl_theta_b'] + inp['pnl_theta_w'] @ cnl_bf)[:, None])
    bias2 = (pnl_bf + cnl_bf)
    put(cf, CF, 'b2', np.stack([bias2[:128], bias2[128:]], axis=1))
    fc1 = inp['ca_fc1_w'].T
    put(cf, CF, 'fc1T', np.concatenate([fc1[:128], fc1[128:]], axis=1))
    put(cf, CF, 'fc2T', inp['ca_fc2_w'].T)
    put(cf, CF, 'onef', np.ones((1, 1), dtype=np.float32))

    f['cbA'] = cbA.astype(ml_dtypes.bfloat16)
    f['cbB'] = cbB.astype(ml_dtypes.bfloat16)
    f['cf'] = cf.astype(np.float32)
    return f


def build_nc(w_fuse):
    nc = bacc.Bacc(None)
    x_d = nc.declare_dram_parameter("x", [128, 2, N], BF16, isOutput=False)
    x0_d = nc.declare_dram_parameter("x0", [128, N], BF16, isOutput=False)
    cbA_d = nc.declare_dram_parameter("cbA", [128, CBA_COLS], BF16, isOutput=False)
    cbB_d = nc.declare_dram_parameter("cbB", [128, CBB_COLS], BF16, isOutput=False)
    cf_d = nc.declare_dram_parameter("cf", [128, CF_COLS], F32R, isOutput=False)
    out_d = nc.declare_dram_parameter("out", [256, N], BF16, isOutput=True)

    with tile.TileContext(nc) as tc:
        _frees = []

        def _keep(pair):
            _frees.append(pair[1])
            return pair[0]

        # ---- persistent SBUF tensors ----
        x_t = _keep(tc.tile([128, 2, N], BF16, name="x_t"))
        x0_t = _keep(tc.tile([128, N], BF16, name="x0_t"))
        cbA_t = _keep(tc.tile([128, CBA_COLS], BF16, name="cbA_t"))
        cbB_t = _keep(tc.tile([128, CBB_COLS], BF16, name="cbB_t"))
        cf_t = _keep(tc.tile([128, CF_COLS], F32R, name="cf_t"))
        x0cat = _keep(tc.tile([128, 32, 256], BF16, name="x0cat"))
        fold1_s = _keep(tc.tile([128, 320], F32R, name="fold1_s"))
        WDC_s = _keep(tc.tile([128, 320], BF16, name="WDC_s"))
        S2_s = _keep(tc.tile([128, 128], BF16, name="S2_s"))
        T2 = _keep(tc.tile([128, M], BF16, name="T2"))
        WS_sb = _keep(tc.tile([128, 512], BF16, name="WS_sb"))
        z_t = _keep(tc.tile([128, 2, N], BF16, name="z_t"))
        bz = _keep(tc.tile([128, 2], F32, name="bz"))
        bT2 = _keep(tc.tile([128, 1], F32, name="bT2"))
        psum_cols = _keep(tc.tile([128, 2, 8], F32, name="psum_cols"))
        macc = _keep(tc.tile([128, 2, 512], BF16, name="macc"))
        V_t = _keep(tc.tile([128, 2, 2], F32, name="V_t"))
        h_t = _keep(tc.tile([16, 2], F32, name="h_t"))
        ca_t = _keep(tc.tile([128, 2], F32, name="ca_t"))
        ca_bf = _keep(tc.tile([128, 2], BF16, name="ca_bf"))
        tmp1 = _keep(tc.tile([128, 4], F32, name="tmp1"))
        xp_t = _keep(tc.tile([128, 2, N], BF16, name="xp_t"))
        tA = _keep(tc.tile([128, N], BF16, name="tA"))
        mx8 = _keep(tc.tile([8, 512], BF16, name="mx8"))
        mapT_meanP = _keep(tc.tile([64, 64], BF16, name="mapT_meanP"))
        mapT_maxP = _keep(tc.tile([64, 64], BF16, name="mapT_maxP"))
        R_sb = _keep(tc.tile([64, 448], BF16, name="R_sb"))
        sig2d = _keep(tc.tile([64, 64], BF16, name="sig2d"))
        sigb = _keep(tc.tile([128, 1, N], BF16, name="sigb"))

        def cA(name, rows=None):
            off, cols, rws = CBA[name]
            return cbA_t[0:(rows or rws), off:off + cols]

        def cB(name, rows=None):
            off, cols, rws = CBB[name]
            return cbB_t[0:(rows or rws), off:off + cols]

        def cF(name, rows=None):
            off, cols, rws = CF[name]
            return cf_t[0:(rows or rws), off:off + cols]

        from contextlib import ExitStack
        stack = ExitStack()

        # ---- DMAs: first pixel group + early consts, then the rest ----
        nc.sync.dma_start(out=x0_t[:, 0:512], in_=x0_d[:, 0:512])
        nc.sync.dma_start(out=x_t[:, :, 0:512], in_=x_d[:, :, 0:512])
        nc.sync.dma_start(out=cbA_t[:, :], in_=cbA_d[:, :])
        nc.sync.dma_start(out=x0_t[:, 512:2048], in_=x0_d[:, 512:2048])
        nc.sync.dma_start(out=x_t[:, :, 512:2048], in_=x_d[:, :, 512:2048])
        nc.sync.dma_start(out=x0_t[:, 2048:4096], in_=x0_d[:, 2048:4096])
        nc.sync.dma_start(out=x_t[:, :, 2048:4096], in_=x_d[:, :, 2048:4096])
        nc.sync.dma_start(out=cbB_t[:, :], in_=cbB_d[:, :])
        nc.sync.dma_start(out=cf_t[:, :], in_=cf_d[:, :])

        sp = stack.enter_context(tc.tile_pool(name="sp", bufs=3))

        # warm the sigmoid act-table set (contains identity/copy/relu too)
        warm = sp.tile([1, 8], F32, tag="warm", name="warm", bufs=1)
        nc.vector.memset(warm[:, :], 0.0)
        nc.scalar.activation(out=warm[:, :], in_=warm[:, :], func=AF.Sigmoid)
        onescol = sp.tile([128, 1], BF16, tag="onescol", name="onescol",
                          bufs=1)
        nc.vector.memset(onescol[:, :], 1.0)

        # xp = (1-w)*x on the otherwise-idle Pool engine (SBUF-only there);
        # each chunk is pre-written to out_d in the idle DMA window so the
        # final add happens via DMA accumulate instead of DVE
        for g in range(4):
            nc.gpsimd.tensor_scalar(out=xp_t[:, :, bass.ts(g, 1024)],
                                    in0=x_t[:, :, bass.ts(g, 1024)],
                                    scalar1=1.0 - w_fuse, scalar2=None,
                                    op0=ALU.mult)
            nc.sync.dma_start(
                out=out_d[:, bass.ts(g, 1024)].rearrange(
                    "(two p) n -> p two n", two=2),
                in_=xp_t[:, :, bass.ts(g, 1024)])

        # =========== Stage A: x0cat + G = x@ph^T (theta never applied
        # per-pixel: att = w_th^T G + b_th (x) colsum(ph)) ===========
        ps1_ctx = tc.tile_pool(name="ps1", bufs=1, space="PSUM")
        ps1 = ps1_ctx.__enter__()
        ps_s = ps1.tile([64, 256], F32, tag="S2", name="ps_s")
        with tc.tile_pool(name="psA", bufs=2, space="PSUM") as psA:
            G_ps = psA.tile([128, 2, 128], F32, tag="G", name="G_ps", bufs=1)
            cs_ps = psA.tile([1, 128], F32, tag="cs", name="cs_ps", bufs=1)
            for t8 in range(8):
                ps_x0c = psA.tile([128, 1024], F32, tag="x0c", name="ps_x0c")
                for sub in range(4):
                    i = 4 * t8 + sub
                    nc.tensor.matmul(ps_x0c[:, bass.ts(sub, 256)],
                                     x0_t[:, bass.ts(i, 128)], cA('w_x0cat'),
                                     start=True, stop=False)
                    # fold the S/Y2-part bias in via a rank-1 matmul so its
                    # drain is a plain copy (Pool cannot read PSUM)
                    nc.tensor.matmul(ps_x0c[:, 256 * sub + 128:
                                             256 * sub + 256],
                                     cA('ones1'), cA('b_x0cat', 1)[:, 128:256],
                                     start=False, stop=True)
                pv = ps_x0c[:, :].rearrange("p (a c) -> p a c", c=256)
                bv = cA('b_x0cat').rearrange("p (a c) -> p a c", c=256)
                # urgent (att) part biased on DVE, lazy part copied on Act
                nc.vector.tensor_tensor(
                    out=x0cat[:, 4 * t8:4 * t8 + 4, 0:128],
                    in0=pv[:, :, 0:128],
                    in1=bv[:, :, 0:128].broadcast_to([128, 4, 128]),
                    op=ALU.add)
                nc.scalar.activation(
                    out=x0cat[:, 4 * t8:4 * t8 + 4, 128:256],
                    in_=pv[:, :, 128:256], func=AF.Copy)
                for sub in range(4):
                    i = 4 * t8 + sub
                    st = (i == 0)
                    sp_ = (i == 31)
                    for ch in range(2):
                        nc.tensor.matmul(G_ps[:, ch, :],
                                         x_t[:, ch, bass.ts(i, 128)],
                                         x0cat[:, i, 0:128],
                                         start=st, stop=sp_)
                    nc.tensor.matmul(cs_ps[:, :], onescol[:, :],
                                     x0cat[:, i, 0:128],
                                     start=st, stop=sp_)
            G_sb = sp.tile([128, 2, 128], BF16, tag="G_sb", name="G_sb",
                           bufs=1)
            nc.vector.tensor_copy(out=G_sb[:, 0, :], in_=G_ps[:, 0, :])
            nc.scalar.activation(out=G_sb[:, 1, :], in_=G_ps[:, 1, :],
                                 func=AF.Copy)
            cs_sb = sp.tile([1, 128], BF16, tag="cs_sb", name="cs_sb", bufs=1)
            nc.vector.tensor_copy(out=cs_sb[:, :], in_=cs_ps[:, :])

        # =========== folds + T + z + channel attention ===========
        with tc.tile_pool(name="psB", bufs=2, space="PSUM") as psB:
            # S blocks transposed (stat=G, mov=P): S2T[g, c]; consecutive
            # emission (interleaving the four shared-bank psum streams with
            # other matmuls corrupts the accumulation)
            for j in range(16):
                st = (j == 0)
                sp_ = (j == 15)
                GTa = x0cat[:, j, 192:256]
                GTb = x0cat[:, j + 16, 192:256]
                Ppair = x0cat[:, j:j + 17:16, 128:192]
                nc.tensor.matmul(ps_s[:, 0:128].rearrange(
                                     "p (a b) -> p a b", a=2),
                                 GTa, Ppair, start=st, stop=sp_)
                nc.tensor.matmul(ps_s[:, 128:256].rearrange(
                                     "p (a b) -> p a b", a=2),
                                 GTb, Ppair, start=st, stop=sp_)
            nc.vector.tensor_copy(out=S2_s[0:64, :], in_=ps_s[:, 0:128])
            nc.vector.tensor_copy(out=S2_s[64:128, :], in_=ps_s[:, 128:256])
            # WS = S2T-halves contracted with w_pnlW: z reads T2 directly
            ps_ws = psB.tile([128, 512], F32, tag="ws", name="ps_ws", bufs=1)
            for h in range(2):
                for ch in range(2):
                    nc.tensor.matmul(ps_ws[:, bass.ts(2 * h + ch, 128)],
                                     S2_s[64 * h:64 * h + 64, :],
                                     cB('w_pnlW')[64 * h:64 * h + 64,
                                                  bass.ts(ch, 128)],
                                     start=True, stop=True)
            nc.vector.tensor_copy(out=WS_sb[:, :], in_=ps_ws[:, :])

            # fold1 = G^T WB + cs (x) bb  -> [wta | WA] (att never built)
            ps_f1 = psB.tile([128, 320], F32, tag="sm", name="ps_f1",
                             bufs=1)
            nc.tensor.matmul(ps_f1[:, :], G_sb[:, 0, :], cB('WB0'),
                             start=True, stop=False)
            nc.tensor.matmul(ps_f1[:, :], G_sb[:, 1, :], cB('WB1'),
                             start=False, stop=False)
            nc.tensor.matmul(ps_f1[:, :], cs_sb[:, :], cB('bb', 1),
                             start=False, stop=True)
            nc.scalar.copy(out=fold1_s[:, :], in_=ps_f1[:, :])
            # fold2 = w_gT @ [wta | WA] -> [WD | WC]
            ps_f2 = psB.tile([128, 320], F32, tag="sm", name="ps_f2",
                             bufs=1)
            nc.tensor.matmul(ps_f2[:, :], _R(cF('w_gT')), fold1_s[:, :],
                             start=True, stop=True)
            nc.vector.tensor_copy(out=WDC_s[:, :], in_=ps_f2[:, :])
            # bT2 = wta^T b_g + b_th2 (row-broadcast over partitions)
            ps_bt = psB.tile([64, 2], F32, tag="sm", name="ps_bt", bufs=1)
            nc.tensor.matmul(ps_bt[:, :], fold1_s[:, 0:64], _R(cF('b_g')),
                             start=True, stop=True)
            nc.vector.tensor_tensor(out=bT2[0:64, :], in0=ps_bt[:, 0:1],
                                    in1=cF('b_th2').bitcast(F32), op=ALU.add)
            nc.vector.tensor_copy(out=bT2[64:128, :], in_=bT2[0:64, :])
            # bz = WA^T b_g + b2
            ps_bb = psB.tile([128, 4], F32, tag="sm", name="ps_bb", bufs=1)
            nc.tensor.matmul(ps_bb[:, 0:2], fold1_s[:, 64:192], _R(cF('b_g')),
                             start=True, stop=True)
            nc.tensor.matmul(ps_bb[:, 2:4], fold1_s[:, 192:320], _R(cF('b_g')),
                             start=True, stop=True)
            nc.vector.tensor_tensor(out=bz[:, 0:1], in0=ps_bb[:, 0:1],
                                    in1=cF('b2')[:, 0:1].bitcast(F32), op=ALU.add)
            nc.vector.tensor_tensor(out=bz[:, 1:2], in0=ps_bb[:, 2:3],
                                    in1=cF('b2')[:, 1:2].bitcast(F32), op=ALU.add)

            # ---- T2 [128, M] interleaved with z emission ----
            def emit_T2(tm):
                ps_T = psB.tile([128, 512], F32, tag="TY", name="ps_T")
                for h in range(2):
                    base = h * M + tm * 512
                    o = ps_T[64 * h:64 * h + 64, :]
                    nc.tensor.matmul(o, cB('w_th2')[:, 0:64],
                                     x_t[:, 0, base:base + 512],
                                     start=True, stop=False)
                    nc.tensor.matmul(o, cB('w_th2')[:, 64:128],
                                     x_t[:, 1, base:base + 512],
                                     start=False, stop=False)
                    nc.tensor.matmul(o, WDC_s[:, 0:64], x0_t[:, base:base + 512],
                                     start=False, stop=True)
                nc.scalar.activation(out=T2[:, bass.ts(tm, 512)], in_=ps_T[:, :],
                                     func=AF.Identity, bias=bT2[:, :])

            for tm in range(4):
                emit_T2(tm)

            # ---- z [128, 2, N] bf16 ----
            for t in range(8):
                h = t // 4
                mbase = (t % 4) * 512
                for ch in range(2):
                    ps_z = psB.tile([128, 512], F32, tag="z", name="ps_z",
                                    bufs=3)
                    nc.tensor.matmul(ps_z[:, :],
                                     WS_sb[:, bass.ts(2 * h + ch, 128)],
                                     T2[:, mbase:mbase + 512],
                                     start=True, stop=False)
                    act_path = (ch == 0 and t < 5) or (ch == 1 and t >= 3)
                    nc.tensor.matmul(ps_z[:, :], WDC_s[:, 64 + 128 * ch:
                                                       192 + 128 * ch],
                                     x0_t[:, bass.ts(t, 512)],
                                     start=False, stop=not act_path)
                    if act_path:
                        nc.tensor.matmul(ps_z[:, :], cA('ident_bf'),
                                         x_t[:, ch, bass.ts(t, 512)],
                                         start=False, stop=True)
                        nc.scalar.activation(
                            out=z_t[:, ch, bass.ts(t, 512)], in_=ps_z[:, :],
                            func=AF.Identity, bias=bz[:, ch:ch + 1],
                            accum_out=psum_cols[:, ch, t:t + 1])
                    else:
                        nc.vector.scalar_tensor_tensor(
                            out=z_t[:, ch, bass.ts(t, 512)], in0=ps_z[:, :],
                            scalar=bz[:, ch:ch + 1],
                            in1=x_t[:, ch, bass.ts(t, 512)], op0=ALU.add,
                            op1=ALU.add,
                            accum_out=psum_cols[:, ch, t:t + 1])
                # rolling channel-wise max: first half's pixel-reduce runs
                # during z production so only half remains on the ca spine
                if t == 0:
                    nc.vector.tensor_copy(out=macc[:, :, :],
                                          in_=z_t[:, :, 0:512])
                elif t == 4:
                    nc.vector.reduce_max(out=V_t[:, :, 1:2],
                                         in_=macc[:, :, :],
                                         axis=mybir.AxisListType.X)
                    nc.vector.tensor_copy(out=macc[:, :, :],
                                          in_=z_t[:, :, 2048:2560])
                else:
                    nc.vector.tensor_tensor(
                        out=macc[:, :, :], in0=macc[:, :, :],
                        in1=z_t[:, :, bass.ts(t, 512)], op=ALU.max)


            # ---- CBAM channel attention (compressed chain) ----
            nc.vector.reduce_max(out=V_t[:, :, 0:1], in_=macc[:, :, :],
                                 axis=mybir.AxisListType.X)
            nc.vector.tensor_tensor(out=V_t[:, :, 1:2], in0=V_t[:, :, 0:1],
                                    in1=V_t[:, :, 1:2], op=ALU.max)
            nc.vector.reduce_sum(out=tmp1[:, 2:4], in_=psum_cols[:, :, :],
                                 axis=mybir.AxisListType.X)
            nc.scalar.activation(out=V_t[:, :, 0:1], in_=tmp1[:, 2:4],
                                 func=AF.Identity, scale=1.0 / float(N))
            ps_f1b = psB.tile([16, 2], F32, tag="sm", name="ps_f1b",
                              bufs=1)
            nc.tensor.matmul(ps_f1b[:, :], cF('fc1T')[:, 0:16].bitcast(F32), V_t[:, 0, :],
                             start=True, stop=False)
            nc.tensor.matmul(ps_f1b[:, :], cF('fc1T')[:, 16:32].bitcast(F32), V_t[:, 1, :],
                             start=False, stop=True)
            nc.scalar.activation(out=h_t[:, :], in_=ps_f1b[:, :], func=AF.Relu)
            ps_f2b = psB.tile([128, 2, 2], F32, tag="sm", name="ps_f2b",
                              bufs=1)
            for ch in range(2):
                nc.tensor.matmul(ps_f2b[:, ch, :],
                                 cF('fc2T')[:, bass.ts(ch, 128)].bitcast(F32),
                                 h_t[:, :], start=True, stop=True)
            nc.vector.reduce_sum(out=tmp1[:, 0:2], in_=ps_f2b[:, :, :],
                                 axis=mybir.AxisListType.X)
            nc.scalar.activation(out=ca_t[:, :], in_=tmp1[:, 0:2],
                                 func=AF.Sigmoid)
            nc.scalar.activation(out=ca_bf[:, :], in_=ca_t[:, :],
                                 func=AF.Copy)

        ps1_ctx.__exit__(None, None, None)

        # =========== maps + sa conv + final ===========
        from concourse import bass_isa
        with tc.tile_pool(name="psC", bufs=2, space="PSUM") as psC:
            # zs = z*ca spread over Act/DVE/Pool; tA + partition-max chase
            # per 1024-chunk; mean mapT built directly from tiny PE matmuls
            # (stationary = z 64-col block, moving = ca column)
            ps_tm = psC.tile([64, 64], F32, tag="tm", name="ps_tm", bufs=1)
            ps_tx = psC.tile([64, 64], F32, tag="tm", name="ps_tx", bufs=1)
            for g in range(4):
                for y in range(16 * g, 16 * g + 16):
                    nc.tensor.matmul(ps_tm[:, y:y + 1],
                                     z_t[:, 0, 64 * y:64 * y + 64],
                                     ca_bf[:, 0:1], start=True, stop=False)
                    nc.tensor.matmul(ps_tm[:, y:y + 1],
                                     z_t[:, 1, 64 * y:64 * y + 64],
                                     ca_bf[:, 1:2], start=False, stop=True)
                for ch in range(2):
                    nc.vector.tensor_scalar(
                        out=z_t[:, ch, bass.ts(g, 1024)],
                        in0=z_t[:, ch, bass.ts(g, 1024)],
                        scalar1=ca_t[:, ch:ch + 1], scalar2=None,
                        op0=ALU.mult)
                # tA = max over the channel pairs; partition-max on Pool
                nc.vector.tensor_tensor(out=tA[:, bass.ts(g, 1024)],
                                        in0=z_t[:, 0, bass.ts(g, 1024)],
                                        in1=z_t[:, 1, bass.ts(g, 1024)],
                                        op=ALU.max)
                mxf = sp.tile([128, 1024], F32, tag="mxf", name="mxf", bufs=2)
                nc.gpsimd.partition_all_reduce(mxf[:, :],
                                               tA[:, bass.ts(g, 1024)], 128,
                                               bass_isa.ReduceOp.max)
                for y in range(16 * g, 16 * g + 16):
                    nc.tensor.transpose(
                        ps_tx[:, y:y + 1],
                        mxf[0:1, 64 * (y - 16 * g):64 * (y - 16 * g) + 64],
                        cF('onef').bitcast(F32))
            nc.vector.tensor_copy(out=mapT_meanP[:, :], in_=ps_tm[:, :])
            nc.scalar.activation(out=mapT_maxP[:, 0:32], in_=ps_tx[:, 0:32],
                                 func=AF.Copy)
            nc.scalar.activation(out=mapT_maxP[:, 32:64], in_=ps_tx[:, 32:64],
                                 func=AF.Copy)

            # sa conv (banded) + sigmoid; max stationary split per半 so the
            # first half accumulates while later preduce chunks still run
            ps_R = psC.tile([64, 448], F32, tag="sm2", name="ps_R")
            nc.tensor.matmul(ps_R[:, :], mapT_meanP[:, :], cB('Kcat2')[:, 0:448],
                             start=True, stop=False)
            nc.tensor.matmul(ps_R[0:32, :], mapT_maxP[:, 0:32],
                             cB('Kcat2')[:, 448:896], start=False, stop=True)
            nc.tensor.matmul(ps_R[32:64, :], mapT_maxP[:, 32:64],
                             cB('Kcat2')[:, 448:896], start=False, stop=True)
            nc.scalar.activation(out=R_sb[:, :], in_=ps_R[:, :], func=AF.Copy)
            ps_sa = psC.tile([64, 64], F32, tag="sm2", name="ps_sa")
            for dy in range(7):
                nc.tensor.matmul(ps_sa[:, :], cB('Sdy7')[:, bass.ts(dy, 64)],
                                 R_sb[:, bass.ts(dy, 64)],
                                 start=(dy == 0), stop=(dy == 6))
            nc.scalar.activation(out=sig2d[:, :], in_=ps_sa[:, :], func=AF.Sigmoid)

            # sigb broadcast straight from sig2d: stationary is a broadcast
            # w_fuse*ident column (selects row y), moving is the whole map
            # out = zs*sigb + xp, per-group pipelined with DMA out
            for t in range(8):
                ps_bc = psC.tile([128, 512], F32, tag="bc", name="ps_bc")
                for yl in range(8):
                    y = 8 * t + yl
                    nc.tensor.matmul(ps_bc[:, bass.ts(yl, 64)],
                                     cB('wident')[:, y:y + 1].broadcast_to(
                                         [64, 128]),
                                     sig2d[:, :], start=True, stop=True)
                nc.scalar.activation(out=sigb[:, 0, bass.ts(t, 512)],
                                     in_=ps_bc[:, :], func=AF.Copy)
                sl = bass.ts(t, 512)
                if t % 2 == 0:
                    vt2 = sp.tile([128, 2, 1024], BF16, tag="vt2",
                                  name="vt2", bufs=4)
                sgb = sigb[:, :, sl].broadcast_to([128, 2, 512])
                nc.vector.tensor_tensor(
                    out=vt2[:, :, 512 * (t % 2):512 * (t % 2) + 512],
                    in0=z_t[:, :, sl], in1=sgb, op=ALU.mult)
                if t % 2 == 1:
                    g = t // 2
                    nc.gpsimd.dma_start(
                        out=out_d[:, bass.ts(g, 1024)].rearrange(
                            "(two p) n -> p two n", two=2),
                        in_=vt2[:, :, :], accum_op=ALU.add)
        stack.close()
        for fr in reversed(_frees):
            fr()
    nc.compile()
    return nc


_CACHE = {}


def kernel(**inputs):
    inp = {k: np.asarray(v) for k, v in inputs.items()}
    f = fold_params(inp)
    key = round(f['w_fuse'], 9)
    if key not in _CACHE:
        _CACHE[key] = build_nc(f['w_fuse'])
    nc = _CACHE[key]

    B = inp['x'].shape[0]
    in_maps = []
    for b in range(B):
        xb = inp['x'][b].reshape(256, N).astype(np.float32)
        m = {
            'x': np.ascontiguousarray(
                xb.reshape(2, 128, N).transpose(1, 0, 2)).astype(ml_dtypes.bfloat16),
            'x0': np.ascontiguousarray(
                inp['x0'][b].reshape(128, N)).astype(ml_dtypes.bfloat16),
            'cbA': f['cbA'], 'cbB': f['cbB'], 'cf': f['cf'],
        }
        in_maps.append(m)

    res = run_bass_kernel_spmd(nc, in_maps, core_ids=list(range(B)))
    out = np.stack([np.asarray(res.results[b]['out'], dtype=np.float32
                               ).reshape(256, H, W) for b in range(B)])
    return out


# revision 54
# speedup vs baseline: 1.0039x; 1.0039x over previous
"""Trainium2 Bass kernel for nn_MDFO (CNL + PNL non-local blocks + CBAM + fusion).

Restructured v7 (pure data-parallel, B=8 over 8 cores, params replicated):
  - bf16 inputs (x, x0) uploaded from host; bf16 output, fp32 on host.
  - all constants packed into three blob DMAs (early-bf16, late-bf16, f32).
  - theta never applied per-pixel and att never materialized: stage A only
    accumulates G = x @ ph^T and colsum(ph); fold1 = G^T WB + cs (x) bb with
    WB = w_th @ [w_tyT|w_cnlW] folded host-side; fold2 = w_gT @ fold1.
  - Y2 never materialized: WS = S2T-halves @ w_pnlW lets z contract T2
    directly; S blocks emitted transposed with paired G columns.
  - z emission split across Act (ident-matmul + bias path) and DVE stt,
    with the CBAM mean accumulated via accum_out and the channel max via
    two rolling chains (first half's pixel reduce runs during z).
  - mean mapT built by tiny per-column matmuls (stationary = z 64-col
    block, moving = ca column); max map via Pool partition_all_reduce into
    replicated f32 rows + tiny f32 column transposes; Sdy has no perm.
  - sigmoid broadcast straight from sig2d via broadcast-stationary
    w*ident-column matmuls (no sigrow extraction).
  - (1-w)*x computed on Pool and pre-written to out_d during the idle DMA
    window; the final z*ca*sigb multiply is accumulated on top with
    software-DGE accum DMAs, eliminating the final add pass entirely.
"""
import sys

import numpy as np

sys.path.insert(0, "/opt/trn_rl_repo")

import ml_dtypes  # noqa: E402

import concourse.bass as bass  # noqa: E402
import concourse.bacc as bacc  # noqa: E402
import concourse.tile as tile  # noqa: E402
from concourse import mybir  # noqa: E402
from concourse.bass_utils import run_bass_kernel_spmd  # noqa: E402

EPS = 1e-5
F32 = mybir.dt.float32
F32R = mybir.dt.float32r
BF16 = mybir.dt.bfloat16
AF = mybir.ActivationFunctionType
ALU = mybir.AluOpType

Ch, Cl, H, W = 256, 128, 64, 64
N = H * W            # 4096
M = N // 2           # 2048
r = Cl // 2          # 64

# blob layouts: name -> (col offset, cols, rows)
CBA_COLS = 768   # early bf16 blob
CBA = {'w_x0cat': (0, 256, 128), 'b_x0cat': (256, 256, 128),
       'ones1': (512, 128, 1), 'ident_bf': (640, 128, 128)}
CBB_COLS = 2752  # late bf16 blob
CBB = {'w_th2': (0, 128, 128), 'w_pnlW': (128, 256, 128),
       'Kcat2': (384, 896, 64), 'Sdy7': (1280, 448, 64),
       'wident': (1728, 64, 64), 'WB0': (1792, 320, 128),
       'WB1': (2112, 320, 128), 'bb': (2432, 320, 1)}
CF_COLS = 422    # f32 blob
CF = {'w_gT': (0, 128, 128), 'b_g': (128, 2, 128),
      'b_th2': (130, 1, 64), 'b2': (131, 2, 128), 'fc1T': (133, 32, 128),
      'fc2T': (165, 256, 16), 'onef': (421, 1, 1)}


def _R(ap):
    return ap.bitcast(F32R)


def fold_params(inp):
    """Host-side constant folding into three blob arrays."""
    f = {}
    scale1 = inp['cnl_bn_g'] / np.sqrt(inp['cnl_bn_v'] + EPS)
    cnl_bf = (inp['cnl_W_b'] * scale1 + inp['cnl_bn_b']
              - inp['cnl_bn_m'] * scale1).astype(np.float32)
    scale2 = inp['pnl_bn_g'] / np.sqrt(inp['pnl_bn_v'] + EPS)
    pnl_bf = (inp['pnl_W_b'] * scale2 + inp['pnl_bn_b']
              - inp['pnl_bn_m'] * scale2).astype(np.float32)
    w_fuse = float(inp['fusion_weight'])
    f['w_fuse'] = w_fuse

    cbA = np.zeros((128, CBA_COLS), dtype=np.float32)
    cbB = np.zeros((128, CBB_COLS), dtype=np.float32)
    cf = np.zeros((128, CF_COLS), dtype=np.float32)

    def put(blob, table, name, arr):
        off, cols, rows = table[name]
        blob[:rows, off:off + cols] = arr

    put(cbA, CBA, 'w_x0cat', np.concatenate([
        inp['cnl_phi_w'].T, inp['pnl_phi_w'].T, (inp['pnl_g_w'] / M).T],
        axis=1))
    brow = np.concatenate([inp['cnl_phi_b'], inp['pnl_phi_b'],
                           inp['pnl_g_b'] / M])
    put(cbA, CBA, 'b_x0cat', np.tile(brow[None, :], (128, 1)))
    put(cbA, CBA, 'ones1', np.ones((1, 128), dtype=np.float32))
    put(cbA, CBA, 'ident_bf', np.eye(128, dtype=np.float32))

    th2 = inp['pnl_theta_w'].T
    put(cbB, CBB, 'w_th2', np.concatenate([th2[:128], th2[128:]], axis=1))
    w_pnlW = (scale2[:, None] * inp['pnl_W_w']).T
    put(cbB, CBB, 'w_pnlW', np.concatenate([w_pnlW, w_pnlW], axis=0))
    # sa conv banded mats; only 1/256 fold on the mean channel (no w folds)
    sa_w = np.asarray(inp['sa_conv_w'][0], dtype=np.float32).copy()
    sa_w[0] /= 256.0
    Kcat = np.zeros((2, 64, 7 * 64), dtype=np.float32)
    for ch in range(2):
        for dy in range(7):
            for dx in range(7):
                w_ = sa_w[ch, dy, dx]
                if w_ == 0.0:
                    continue
                for x in range(64):
                    xq = x + dx - 3
                    if 0 <= xq < 64:
                        Kcat[ch, xq, dy * 64 + x] = w_
    put(cbB, CBB, 'Kcat2', np.concatenate([Kcat[0], Kcat[1]], axis=1))
    Sdy = np.zeros((64, 7 * 64), dtype=np.float32)
    for dy in range(7):
        for y in range(64):
            yp = y + dy - 3
            if 0 <= yp < 64:
                Sdy[yp, dy * 64 + y] = 1.0
    put(cbB, CBB, 'Sdy7', Sdy)
    put(cbB, CBB, 'wident', w_fuse * np.eye(64, dtype=np.float32))
    # fold theta through the fold-1 weights: fold1 = G^T WB + cs (x) bb
    w_big = np.concatenate([
        (inp['pnl_theta_w'] @ (scale1[:, None] * inp['cnl_W_w'])).T,
        (scale1[:, None] * inp['cnl_W_w']).T], axis=1)
    WB = inp['cnl_theta_w'].T @ w_big
    put(cbB, CBB, 'WB0', WB[:128])
    put(cbB, CBB, 'WB1', WB[128:])
    put(cbB, CBB, 'bb', (inp['cnl_theta_b'] @ w_big)[None, :])

    put(cf, CF, 'w_gT', inp['cnl_g_w'] / Cl)
    bgc = (inp['cnl_g_b'] / Cl)[:, None]
    put(cf, CF, 'b_g', np.concatenate([bgc, bgc], axis=1))
    put(cf, CF, 'b_th2', (inp['pnl_theta_b'] + inp['pnl_theta_w'] @ cnl_bf)[:, None])
    bias2 = (pnl_bf + cnl_bf)
    put(cf, CF, 'b2', np.stack([bias2[:128], bias2[128:]], axis=1))
    fc1 = inp['ca_fc1_w'].T
    put(cf, CF, 'fc1T', np.concatenate([fc1[:128], fc1[128:]], axis=1))
    put(cf, CF, 'fc2T', inp['ca_fc2_w'].T)
    put(cf, CF, 'onef', np.ones((1, 1), dtype=np.float32))

    f['cbA'] = cbA.astype(ml_dtypes.bfloat16)
    f['cbB'] = cbB.astype(ml_dtypes.bfloat16)
    f['cf'] = cf.astype(np.float32)
    return f


def build_nc(w_fuse):
    nc = bacc.Bacc(None)
    x_d = nc.declare_dram_parameter("x", [128, 2, N], BF16, isOutput=False)
    x0_d = nc.declare_dram_parameter("x0", [128, N], BF16, isOutput=False)
    cbA_d = nc.declare_dram_parameter("cbA", [128, CBA_COLS], BF16, isOutput=False)
    cbB_d = nc.declare_dram_parameter("cbB", [128, CBB_COLS], BF16, isOutput=False)
    cf_d = nc.declare_dram_parameter("cf", [128, CF_COLS], F32R, isOutput=False)
    out_d = nc.declare_dram_parameter("out", [256, N], BF16, isOutput=True)

    with tile.TileContext(nc) as tc:
        _frees = []

        def _keep(pair):
            _frees.append(pair[1])
            return pair[0]

        # ---- persistent SBUF tensors ----
        x_t = _keep(tc.tile([128, 2, N], BF16, name="x_t"))
        x0_t = _keep(tc.tile([128, N], BF16, name="x0_t"))
        cbA_t = _keep(tc.tile([128, CBA_COLS], BF16, name="cbA_t"))
        cbB_t = _keep(tc.tile([128, CBB_COLS], BF16, name="cbB_t"))
        cf_t = _keep(tc.tile([128, CF_COLS], F32R, name="cf_t"))
        x0cat = _keep(tc.tile([128, 32, 256], BF16, name="x0cat"))
        fold1_s = _keep(tc.tile([128, 320], F32R, name="fold1_s"))
        WDC_s = _keep(tc.tile([128, 320], BF16, name="WDC_s"))
        S2_s = _keep(tc.tile([128, 128], BF16, name="S2_s"))
        T2 = _keep(tc.tile([128, M], BF16, name="T2"))
        WS_sb = _keep(tc.tile([128, 512], BF16, name="WS_sb"))
        z_t = _keep(tc.tile([128, 2, N], BF16, name="z_t"))
        bz = _keep(tc.tile([128, 2], F32, name="bz"))
        bT2 = _keep(tc.tile([128, 1], F32, name="bT2"))
        psum_cols = _keep(tc.tile([128, 2, 8], F32, name="psum_cols"))
        macc = _keep(tc.tile([128, 2, 512], BF16, name="macc"))
        V_t = _keep(tc.tile([128, 2, 2], F32, name="V_t"))
        h_t = _keep(tc.tile([16, 2], F32, name="h_t"))
        ca_t = _keep(tc.tile([128, 2], F32, name="ca_t"))
        ca_bf = _keep(tc.tile([128, 2], BF16, name="ca_bf"))
        tmp1 = _keep(tc.tile([128, 4], F32, name="tmp1"))
        xp_t = _keep(tc.tile([128, 2, N], BF16, name="xp_t"))
        tA = _keep(tc.tile([128, N], BF16, name="tA"))
        mapT_meanP = _keep(tc.tile([64, 64], BF16, name="mapT_meanP"))
        mapT_maxP = _keep(tc.tile([64, 64], BF16, name="mapT_maxP"))
        R_sb = _keep(tc.tile([64, 448], BF16, name="R_sb"))
        sig2d = _keep(tc.tile([64, 64], BF16, name="sig2d"))
        sigb = _keep(tc.tile([128, 1, N], BF16, name="sigb"))

        def cA(name, rows=None):
            off, cols, rws = CBA[name]
            return cbA_t[0:(rows or rws), off:off + cols]

        def cB(name, rows=None):
            off, cols, rws = CBB[name]
            return cbB_t[0:(rows or rws), off:off + cols]

        def cF(name, rows=None):
            off, cols, rws = CF[name]
            return cf_t[0:(rows or rws), off:off + cols]

        from contextlib import ExitStack
        stack = ExitStack()

        # ---- DMAs: first pixel group + early consts, then the rest ----
        nc.sync.dma_start(out=x0_t[:, 0:512], in_=x0_d[:, 0:512])
        nc.sync.dma_start(out=x_t[:, :, 0:512], in_=x_d[:, :, 0:512])
        nc.sync.dma_start(out=cbA_t[:, :], in_=cbA_d[:, :])
        nc.sync.dma_start(out=x0_t[:, 512:2048], in_=x0_d[:, 512:2048])
        nc.sync.dma_start(out=x_t[:, :, 512:2048], in_=x_d[:, :, 512:2048])
        nc.sync.dma_start(out=x0_t[:, 2048:4096], in_=x0_d[:, 2048:4096])
        nc.sync.dma_start(out=x_t[:, :, 2048:4096], in_=x_d[:, :, 2048:4096])
        nc.sync.dma_start(out=cbB_t[:, :], in_=cbB_d[:, :])
        nc.sync.dma_start(out=cf_t[:, :], in_=cf_d[:, :])

        sp = stack.enter_context(tc.tile_pool(name="sp", bufs=3))

        # warm the sigmoid act-table set (contains identity/copy/relu too)
        warm = sp.tile([1, 8], F32, tag="warm", name="warm", bufs=1)
        nc.vector.memset(warm[:, :], 0.0)
        nc.scalar.activation(out=warm[:, :], in_=warm[:, :], func=AF.Sigmoid)
        onescol = sp.tile([128, 1], BF16, tag="onescol", name="onescol",
                          bufs=1)
        nc.vector.memset(onescol[:, :], 1.0)

        # xp = (1-w)*x on the otherwise-idle Pool engine (SBUF-only there);
        # each chunk is pre-written to out_d in the idle DMA window so the
        # final add happens via DMA accumulate instead of DVE
        for g in range(4):
            nc.gpsimd.tensor_scalar(out=xp_t[:, :, bass.ts(g, 1024)],
                                    in0=x_t[:, :, bass.ts(g, 1024)],
                                    scalar1=1.0 - w_fuse, scalar2=None,
                                    op0=ALU.mult)
            nc.sync.dma_start(
                out=out_d[:, bass.ts(g, 1024)].rearrange(
                    "(two p) n -> p two n", two=2),
                in_=xp_t[:, :, bass.ts(g, 1024)])

        # =========== Stage A: x0cat + G = x@ph^T (theta never applied
        # per-pixel: att = w_th^T G + b_th (x) colsum(ph)) ===========
        ps1_ctx = tc.tile_pool(name="ps1", bufs=1, space="PSUM")
        ps1 = ps1_ctx.__enter__()
        ps_s = ps1.tile([64, 256], F32, tag="S2", name="ps_s")
        with tc.tile_pool(name="psA", bufs=2, space="PSUM") as psA:
            G_ps = psA.tile([128, 2, 128], F32, tag="G", name="G_ps", bufs=1)
            cs_ps = psA.tile([1, 128], F32, tag="cs", name="cs_ps", bufs=1)
            for t8 in range(8):
                ps_x0c = psA.tile([128, 1024], F32, tag="x0c", name="ps_x0c")
                for sub in range(4):
                    i = 4 * t8 + sub
                    nc.tensor.matmul(ps_x0c[:, bass.ts(sub, 256)],
                                     x0_t[:, bass.ts(i, 128)], cA('w_x0cat'),
                                     start=True, stop=False)
                    # fold the S/Y2-part bias in via a rank-1 matmul so its
                    # drain is a plain copy (Pool cannot read PSUM)
                    nc.tensor.matmul(ps_x0c[:, 256 * sub + 128:
                                             256 * sub + 256],
                                     cA('ones1'), cA('b_x0cat', 1)[:, 128:256],
                                     start=False, stop=True)
                pv = ps_x0c[:, :].rearrange("p (a c) -> p a c", c=256)
                bv = cA('b_x0cat').rearrange("p (a c) -> p a c", c=256)
                # urgent (att) part biased on DVE, lazy part copied on Act
                nc.vector.tensor_tensor(
                    out=x0cat[:, 4 * t8:4 * t8 + 4, 0:128],
                    in0=pv[:, :, 0:128],
                    in1=bv[:, :, 0:128].broadcast_to([128, 4, 128]),
                    op=ALU.add)
                nc.scalar.activation(
                    out=x0cat[:, 4 * t8:4 * t8 + 4, 128:256],
                    in_=pv[:, :, 128:256], func=AF.Copy)
                for sub in range(4):
                    i = 4 * t8 + sub
                    st = (i == 0)
                    sp_ = (i == 31)
                    for ch in range(2):
                        nc.tensor.matmul(G_ps[:, ch, :],
                                         x_t[:, ch, bass.ts(i, 128)],
                                         x0cat[:, i, 0:128],
                                         start=st, stop=sp_)
                    nc.tensor.matmul(cs_ps[:, :], onescol[:, :],
                                     x0cat[:, i, 0:128],
                                     start=st, stop=sp_)
            G_sb = sp.tile([128, 2, 128], BF16, tag="G_sb", name="G_sb",
                           bufs=1)
            nc.vector.tensor_copy(out=G_sb[:, 0, :], in_=G_ps[:, 0, :])
            nc.scalar.activation(out=G_sb[:, 1, :], in_=G_ps[:, 1, :],
                                 func=AF.Copy)
            cs_sb = sp.tile([1, 128], BF16, tag="cs_sb", name="cs_sb", bufs=1)
            nc.vector.tensor_copy(out=cs_sb[:, :], in_=cs_ps[:, :])

        # =========== folds + T + z + channel attention ===========
        with tc.tile_pool(name="psB", bufs=2, space="PSUM") as psB:
            # S blocks transposed (stat=G, mov=P): S2T[g, c]; consecutive
            # emission (interleaving the four shared-bank psum streams with
            # other matmuls corrupts the accumulation)
            for j in range(16):
                st = (j == 0)
                sp_ = (j == 15)
                GTa = x0cat[:, j, 192:256]
                GTb = x0cat[:, j + 16, 192:256]
                Ppair = x0cat[:, j:j + 17:16, 128:192]
                nc.tensor.matmul(ps_s[:, 0:128].rearrange(
                                     "p (a b) -> p a b", a=2),
                                 GTa, Ppair, start=st, stop=sp_)
                nc.tensor.matmul(ps_s[:, 128:256].rearrange(
                                     "p (a b) -> p a b", a=2),
                                 GTb, Ppair, start=st, stop=sp_)
            nc.vector.tensor_copy(out=S2_s[0:64, :], in_=ps_s[:, 0:128])
            nc.vector.tensor_copy(out=S2_s[64:128, :], in_=ps_s[:, 128:256])
            # fold1 = G^T WB + cs (x) bb  -> [wta | WA] (att never built)
            ps_f1 = psB.tile([128, 320], F32, tag="sm", name="ps_f1",
                             bufs=1)
            nc.tensor.matmul(ps_f1[:, :], G_sb[:, 0, :], cB('WB0'),
                             start=True, stop=False)
            nc.tensor.matmul(ps_f1[:, :], G_sb[:, 1, :], cB('WB1'),
                             start=False, stop=False)
            nc.tensor.matmul(ps_f1[:, :], cs_sb[:, :], cB('bb', 1),
                             start=False, stop=True)
            nc.scalar.copy(out=fold1_s[:, :], in_=ps_f1[:, :])
            # fold2 = w_gT @ [wta | WA] -> [WD | WC]
            ps_f2 = psB.tile([128, 320], F32, tag="sm", name="ps_f2",
                             bufs=1)
            nc.tensor.matmul(ps_f2[:, :], _R(cF('w_gT')), fold1_s[:, :],
                             start=True, stop=True)
            nc.vector.tensor_copy(out=WDC_s[:, :], in_=ps_f2[:, :])
            # bT2 = wta^T b_g + b_th2 (row-broadcast over partitions)
            ps_bt = psB.tile([64, 2], F32, tag="sm", name="ps_bt", bufs=1)
            nc.tensor.matmul(ps_bt[:, :], fold1_s[:, 0:64], _R(cF('b_g')),
                             start=True, stop=True)
            nc.vector.tensor_tensor(out=bT2[0:64, :], in0=ps_bt[:, 0:1],
                                    in1=cF('b_th2').bitcast(F32), op=ALU.add)
            nc.vector.tensor_copy(out=bT2[64:128, :], in_=bT2[0:64, :])
            # bz = WA^T b_g + b2
            ps_bb = psB.tile([128, 4], F32, tag="sm", name="ps_bb", bufs=1)
            nc.tensor.matmul(ps_bb[:, 0:2], fold1_s[:, 64:192], _R(cF('b_g')),
                             start=True, stop=True)
            nc.tensor.matmul(ps_bb[:, 2:4], fold1_s[:, 192:320], _R(cF('b_g')),
                             start=True, stop=True)
            nc.vector.tensor_tensor(out=bz[:, 0:1], in0=ps_bb[:, 0:1],
                                    in1=cF('b2')[:, 0:1].bitcast(F32), op=ALU.add)
            nc.vector.tensor_tensor(out=bz[:, 1:2], in0=ps_bb[:, 2:3],
                                    in1=cF('b2')[:, 1:2].bitcast(F32), op=ALU.add)

            # WS = S2T-halves contracted with w_pnlW: z reads T2 directly
            ps_ws = psB.tile([128, 512], F32, tag="ws", name="ps_ws", bufs=1)
            for h in range(2):
                for ch in range(2):
                    nc.tensor.matmul(ps_ws[:, bass.ts(2 * h + ch, 128)],
                                     S2_s[64 * h:64 * h + 64, :],
                                     cB('w_pnlW')[64 * h:64 * h + 64,
                                                  bass.ts(ch, 128)],
                                     start=True, stop=True)
            nc.vector.tensor_copy(out=WS_sb[:, :], in_=ps_ws[:, :])

            # ---- T2 [128, M] interleaved with z emission ----
            def emit_T2(tm):
                ps_T = psB.tile([128, 512], F32, tag="TY", name="ps_T")
                for h in range(2):
                    base = h * M + tm * 512
                    o = ps_T[64 * h:64 * h + 64, :]
                    nc.tensor.matmul(o, cB('w_th2')[:, 0:64],
                                     x_t[:, 0, base:base + 512],
                                     start=True, stop=False)
                    nc.tensor.matmul(o, cB('w_th2')[:, 64:128],
                                     x_t[:, 1, base:base + 512],
                                     start=False, stop=False)
                    nc.tensor.matmul(o, WDC_s[:, 0:64], x0_t[:, base:base + 512],
                                     start=False, stop=True)
                nc.scalar.activation(out=T2[:, bass.ts(tm, 512)], in_=ps_T[:, :],
                                     func=AF.Identity, bias=bT2[:, :])

            for tm in range(4):
                emit_T2(tm)

            # ---- z [128, 2, N] bf16 ----
            for t in range(8):
                h = t // 4
                mbase = (t % 4) * 512
                for ch in range(2):
                    ps_z = psB.tile([128, 512], F32, tag="z", name="ps_z",
                                    bufs=3)
                    nc.tensor.matmul(ps_z[:, :],
                                     WS_sb[:, bass.ts(2 * h + ch, 128)],
                                     T2[:, mbase:mbase + 512],
                                     start=True, stop=False)
                    act_path = (ch == 0 and t < 5) or (ch == 1 and t >= 3)
                    nc.tensor.matmul(ps_z[:, :], WDC_s[:, 64 + 128 * ch:
                                                       192 + 128 * ch],
                                     x0_t[:, bass.ts(t, 512)],
                                     start=False, stop=not act_path)
                    if act_path:
                        nc.tensor.matmul(ps_z[:, :], cA('ident_bf'),
                                         x_t[:, ch, bass.ts(t, 512)],
                                         start=False, stop=True)
                        nc.scalar.activation(
                            out=z_t[:, ch, bass.ts(t, 512)], in_=ps_z[:, :],
                            func=AF.Identity, bias=bz[:, ch:ch + 1],
                            accum_out=psum_cols[:, ch, t:t + 1])
                    else:
                        nc.vector.scalar_tensor_tensor(
                            out=z_t[:, ch, bass.ts(t, 512)], in0=ps_z[:, :],
                            scalar=bz[:, ch:ch + 1],
                            in1=x_t[:, ch, bass.ts(t, 512)], op0=ALU.add,
                            op1=ALU.add,
                            accum_out=psum_cols[:, ch, t:t + 1])
                # rolling channel-wise max: first half's pixel-reduce runs
                # during z production so only half remains on the ca spine
                if t == 0:
                    nc.vector.tensor_copy(out=macc[:, :, :],
                                          in_=z_t[:, :, 0:512])
                elif t == 4:
                    nc.vector.reduce_max(out=V_t[:, :, 1:2],
                                         in_=macc[:, :, :],
                                         axis=mybir.AxisListType.X)
                    nc.vector.tensor_copy(out=macc[:, :, :],
                                          in_=z_t[:, :, 2048:2560])
                else:
                    nc.vector.tensor_tensor(
                        out=macc[:, :, :], in0=macc[:, :, :],
                        in1=z_t[:, :, bass.ts(t, 512)], op=ALU.max)


            # ---- CBAM channel attention (compressed chain) ----
            nc.vector.reduce_max(out=V_t[:, :, 0:1], in_=macc[:, :, :],
                                 axis=mybir.AxisListType.X)
            nc.vector.tensor_tensor(out=V_t[:, :, 1:2], in0=V_t[:, :, 0:1],
                                    in1=V_t[:, :, 1:2], op=ALU.max)
            nc.vector.reduce_sum(out=tmp1[:, 2:4], in_=psum_cols[:, :, :],
                                 axis=mybir.AxisListType.X)
            nc.scalar.activation(out=V_t[:, :, 0:1], in_=tmp1[:, 2:4],
                                 func=AF.Identity, scale=1.0 / float(N))
            ps_f1b = psB.tile([16, 2], F32, tag="sm", name="ps_f1b",
                              bufs=1)
            nc.tensor.matmul(ps_f1b[:, :], cF('fc1T')[:, 0:16].bitcast(F32), V_t[:, 0, :],
                             start=True, stop=False)
            nc.tensor.matmul(ps_f1b[:, :], cF('fc1T')[:, 16:32].bitcast(F32), V_t[:, 1, :],
                             start=False, stop=True)
            nc.scalar.activation(out=h_t[:, :], in_=ps_f1b[:, :], func=AF.Relu)
            ps_f2b = psB.tile([128, 2, 2], F32, tag="sm", name="ps_f2b",
                              bufs=1)
            for ch in range(2):
                nc.tensor.matmul(ps_f2b[:, ch, :],
                                 cF('fc2T')[:, bass.ts(ch, 128)].bitcast(F32),
                                 h_t[:, :], start=True, stop=True)
            nc.vector.reduce_sum(out=tmp1[:, 0:2], in_=ps_f2b[:, :, :],
                                 axis=mybir.AxisListType.X)
            nc.scalar.activation(out=ca_t[:, :], in_=tmp1[:, 0:2],
                                 func=AF.Sigmoid)
            nc.scalar.activation(out=ca_bf[:, :], in_=ca_t[:, :],
                                 func=AF.Copy)

        ps1_ctx.__exit__(None, None, None)

        # =========== maps + sa conv + final ===========
        from concourse import bass_isa
        with tc.tile_pool(name="psC", bufs=2, space="PSUM") as psC:
            # zs = z*ca spread over Act/DVE/Pool; tA + partition-max chase
            # per 1024-chunk; mean mapT built directly from tiny PE matmuls
            # (stationary = z 64-col block, moving = ca column)
            ps_tm = psC.tile([64, 64], F32, tag="tm", name="ps_tm", bufs=1)
            ps_tx = psC.tile([64, 64], F32, tag="tm", name="ps_tx", bufs=1)
            for g in range(4):
                for y in range(16 * g, 16 * g + 16):
                    nc.tensor.matmul(ps_tm[:, y:y + 1],
                                     z_t[:, 0, 64 * y:64 * y + 64],
                                     ca_bf[:, 0:1], start=True, stop=False)
                    nc.tensor.matmul(ps_tm[:, y:y + 1],
                                     z_t[:, 1, 64 * y:64 * y + 64],
                                     ca_bf[:, 1:2], start=False, stop=True)
                for ch in range(2):
                    nc.vector.tensor_scalar(
                        out=z_t[:, ch, bass.ts(g, 1024)],
                        in0=z_t[:, ch, bass.ts(g, 1024)],
                        scalar1=ca_t[:, ch:ch + 1], scalar2=None,
                        op0=ALU.mult)
                # tA = max over the channel pairs; partition-max on Pool
                nc.vector.tensor_tensor(out=tA[:, bass.ts(g, 1024)],
                                        in0=z_t[:, 0, bass.ts(g, 1024)],
                                        in1=z_t[:, 1, bass.ts(g, 1024)],
                                        op=ALU.max)
                mxf = sp.tile([128, 1024], F32, tag="mxf", name="mxf", bufs=2)
                nc.gpsimd.partition_all_reduce(mxf[:, :],
                                               tA[:, bass.ts(g, 1024)], 128,
                                               bass_isa.ReduceOp.max)
                for y in range(16 * g, 16 * g + 16):
                    nc.tensor.transpose(
                        ps_tx[:, y:y + 1],
                        mxf[0:1, 64 * (y - 16 * g):64 * (y - 16 * g) + 64],
                        cF('onef').bitcast(F32))
            nc.vector.tensor_copy(out=mapT_meanP[:, :], in_=ps_tm[:, :])
            nc.vector.tensor_copy(out=mapT_maxP[:, 0:32], in_=ps_tx[:, 0:32])
            nc.vector.tensor_copy(out=mapT_maxP[:, 32:64],
                                  in_=ps_tx[:, 32:64])

            # sa conv (banded) + sigmoid; max stationary split per半 so the
            # first half accumulates while later preduce chunks still run
            ps_R = psC.tile([64, 448], F32, tag="sm2", name="ps_R")
            nc.tensor.matmul(ps_R[:, :], mapT_meanP[:, :], cB('Kcat2')[:, 0:448],
                             start=True, stop=False)
            nc.tensor.matmul(ps_R[0:32, :], mapT_maxP[:, 0:32],
                             cB('Kcat2')[:, 448:896], start=False, stop=True)
            nc.tensor.matmul(ps_R[32:64, :], mapT_maxP[:, 32:64],
                             cB('Kcat2')[:, 448:896], start=False, stop=True)
            nc.vector.tensor_copy(out=R_sb[:, :], in_=ps_R[:, :])
            ps_sa = psC.tile([64, 64], F32, tag="sm2", name="ps_sa")
            for dy in range(7):
                nc.tensor.matmul(ps_sa[:, :], cB('Sdy7')[:, bass.ts(dy, 64)],
                                 R_sb[:, bass.ts(dy, 64)],
                                 start=(dy == 0), stop=(dy == 6))
            nc.scalar.activation(out=sig2d[:, :], in_=ps_sa[:, :], func=AF.Sigmoid)

            # sigb broadcast straight from sig2d: stationary is a broadcast
            # w_fuse*ident column (selects row y), moving is the whole map
            # out = zs*sigb + xp, per-group pipelined with DMA out
            for t in range(8):
                ps_bc = psC.tile([128, 512], F32, tag="bc", name="ps_bc")
                for yl in range(8):
                    y = 8 * t + yl
                    nc.tensor.matmul(ps_bc[:, bass.ts(yl, 64)],
                                     cB('wident')[:, y:y + 1].broadcast_to(
                                         [64, 128]),
                                     sig2d[:, :], start=True, stop=True)
                nc.scalar.activation(out=sigb[:, 0, bass.ts(t, 512)],
                                     in_=ps_bc[:, :], func=AF.Copy)
                sl = bass.ts(t, 512)
                if t % 2 == 0:
                    vt2 = sp.tile([128, 2, 1024], BF16, tag="vt2",
                                  name="vt2", bufs=4)
                sgb = sigb[:, :, sl].broadcast_to([128, 2, 512])
                nc.vector.tensor_tensor(
                    out=vt2[:, :, 512 * (t % 2):512 * (t % 2) + 512],
                    in0=z_t[:, :, sl], in1=sgb, op=ALU.mult)
                if t % 2 == 1:
                    g = t // 2
                    nc.gpsimd.dma_start(
                        out=out_d[:, bass.ts(g, 1024)].rearrange(
                            "(two p) n -> p two n", two=2),
                        in_=vt2[:, :, :], accum_op=ALU.add)
        stack.close()
        for fr in reversed(_frees):
            fr()
    nc.compile()
    return nc


_CACHE = {}


def kernel(**inputs):
    inp = {k: np.asarray(v) for k, v in inputs.items()}
    f = fold_params(inp)
    key = round(f['w_fuse'], 9)
    if key not in _CACHE:
        _CACHE[key] = build_nc(f['w_fuse'])
    nc = _CACHE[key]

    B = inp['x'].shape[0]
    in_maps = []
    for b in range(B):
        xb = inp['x'][b].reshape(256, N).astype(np.float32)
        m = {
            'x': np.ascontiguousarray(
                xb.reshape(2, 128, N).transpose(1, 0, 2)).astype(ml_dtypes.bfloat16),
            'x0': np.ascontiguousarray(
                inp['x0'][b].reshape(128, N)).astype(ml_dtypes.bfloat16),
            'cbA': f['cbA'], 'cbB': f['cbB'], 'cf': f['cf'],
        }
        in_maps.append(m)

    res = run_bass_kernel_spmd(nc, in_maps, core_ids=list(range(B)))
    out = np.stack([np.asarray(res.results[b]['out'], dtype=np.float32
                               ).reshape(256, H, W) for b in range(B)])
    return out


# revision 57
# speedup vs baseline: 1.0103x; 1.0064x over previous
"""Trainium2 Bass kernel for nn_MDFO (CNL + PNL non-local blocks + CBAM + fusion).

Restructured v7 (pure data-parallel, B=8 over 8 cores, params replicated):
  - bf16 inputs (x, x0) uploaded from host; bf16 output, fp32 on host.
  - all constants packed into three blob DMAs (early-bf16, late-bf16, f32).
  - theta never applied per-pixel and att never materialized: stage A only
    accumulates G = x @ ph^T and colsum(ph); fold1 = G^T WB + cs (x) bb with
    WB = w_th @ [w_tyT|w_cnlW] folded host-side; fold2 = w_gT @ fold1.
  - Y2 never materialized: WS = S2T-halves @ w_pnlW lets z contract T2
    directly; S blocks emitted transposed with paired G columns.
  - z emission split across Act (ident-matmul + bias path) and DVE stt,
    with the CBAM mean accumulated via accum_out and the channel max via
    two rolling chains (first half's pixel reduce runs during z).
  - mean mapT built by tiny per-column matmuls (stationary = z 64-col
    block, moving = ca column); max map via Pool partition_all_reduce into
    replicated f32 rows + tiny f32 column transposes; Sdy has no perm.
  - sigmoid broadcast straight from sig2d via broadcast-stationary
    w*ident-column matmuls (no sigrow extraction).
  - (1-w)*x computed on Pool and pre-written to out_d during the idle DMA
    window; the final z*ca*sigb multiply is accumulated on top with
    software-DGE accum DMAs, eliminating the final add pass entirely.
"""
import sys

import numpy as np

sys.path.insert(0, "/opt/trn_rl_repo")

import ml_dtypes  # noqa: E402

import concourse.bass as bass  # noqa: E402
import concourse.bacc as bacc  # noqa: E402
import concourse.tile as tile  # noqa: E402
from concourse import mybir  # noqa: E402
from concourse.bass_utils import run_bass_kernel_spmd  # noqa: E402

EPS = 1e-5
F32 = mybir.dt.float32
F32R = mybir.dt.float32r
BF16 = mybir.dt.bfloat16
AF = mybir.ActivationFunctionType
ALU = mybir.AluOpType

Ch, Cl, H, W = 256, 128, 64, 64
N = H * W            # 4096
M = N // 2           # 2048
r = Cl // 2          # 64

# blob layouts: name -> (col offset, cols, rows)
CBA_COLS = 768   # early bf16 blob
CBA = {'w_x0cat': (0, 256, 128), 'b_x0cat': (256, 256, 128),
       'ones1': (512, 128, 1), 'ident_bf': (640, 128, 128)}
CBB_COLS = 2374  # late bf16 blob
CBB = {'w_th2': (0, 128, 128), 'w_pnlW': (128, 256, 128),
       'Kcat2': (384, 896, 64), 'identS': (1280, 70, 64),
       'wident': (1350, 64, 64), 'WB0': (1414, 320, 128),
       'WB1': (1734, 320, 128), 'bb': (2054, 320, 1)}
CF_COLS = 454    # f32 blob
CF = {'w_gT': (0, 128, 128), 'b_g': (128, 2, 128),
      'b_th2': (130, 1, 64), 'b2': (131, 2, 128), 'fc1T': (133, 32, 128),
      'fc2T': (165, 256, 16), 'onef': (421, 1, 1), 'fc1TN': (422, 32, 128)}


def _R(ap):
    return ap.bitcast(F32R)


def fold_params(inp):
    """Host-side constant folding into three blob arrays."""
    f = {}
    scale1 = inp['cnl_bn_g'] / np.sqrt(inp['cnl_bn_v'] + EPS)
    cnl_bf = (inp['cnl_W_b'] * scale1 + inp['cnl_bn_b']
              - inp['cnl_bn_m'] * scale1).astype(np.float32)
    scale2 = inp['pnl_bn_g'] / np.sqrt(inp['pnl_bn_v'] + EPS)
    pnl_bf = (inp['pnl_W_b'] * scale2 + inp['pnl_bn_b']
              - inp['pnl_bn_m'] * scale2).astype(np.float32)
    w_fuse = float(inp['fusion_weight'])
    f['w_fuse'] = w_fuse

    cbA = np.zeros((128, CBA_COLS), dtype=np.float32)
    cbB = np.zeros((128, CBB_COLS), dtype=np.float32)
    cf = np.zeros((128, CF_COLS), dtype=np.float32)

    def put(blob, table, name, arr):
        off, cols, rows = table[name]
        blob[:rows, off:off + cols] = arr

    put(cbA, CBA, 'w_x0cat', np.concatenate([
        inp['cnl_phi_w'].T, inp['pnl_phi_w'].T, (inp['pnl_g_w'] / M).T],
        axis=1))
    brow = np.concatenate([inp['cnl_phi_b'], inp['pnl_phi_b'],
                           inp['pnl_g_b'] / M])
    put(cbA, CBA, 'b_x0cat', np.tile(brow[None, :], (128, 1)))
    put(cbA, CBA, 'ones1', np.ones((1, 128), dtype=np.float32))
    put(cbA, CBA, 'ident_bf', np.eye(128, dtype=np.float32))

    th2 = inp['pnl_theta_w'].T
    put(cbB, CBB, 'w_th2', np.concatenate([th2[:128], th2[128:]], axis=1))
    w_pnlW = (scale2[:, None] * inp['pnl_W_w']).T
    put(cbB, CBB, 'w_pnlW', np.concatenate([w_pnlW, w_pnlW], axis=0))
    # sa conv banded mats; only 1/256 fold on the mean channel (no w folds)
    sa_w = np.asarray(inp['sa_conv_w'][0], dtype=np.float32).copy()
    sa_w[0] /= 256.0
    Kcat = np.zeros((2, 64, 7 * 64), dtype=np.float32)
    for ch in range(2):
        for dy in range(7):
            for dx in range(7):
                w_ = sa_w[ch, dy, dx]
                if w_ == 0.0:
                    continue
                for x in range(64):
                    xq = x + dx - 3
                    if 0 <= xq < 64:
                        Kcat[ch, xq, dy * 64 + x] = w_
    put(cbB, CBB, 'Kcat2', np.concatenate([Kcat[0], Kcat[1]], axis=1))
    # shifted identity: identS[p, c] = 1 iff p == c-3; slices [dy:dy+64]
    # reproduce the Sdy one-hot selector blocks of the banded sa conv
    identS = np.zeros((64, 70), dtype=np.float32)
    for c in range(3, 67):
        identS[c - 3, c] = 1.0
    put(cbB, CBB, 'identS', identS)
    put(cbB, CBB, 'wident', w_fuse * np.eye(64, dtype=np.float32))
    # fold theta through the fold-1 weights: fold1 = G^T WB + cs (x) bb
    w_big = np.concatenate([
        (inp['pnl_theta_w'] @ (scale1[:, None] * inp['cnl_W_w'])).T,
        (scale1[:, None] * inp['cnl_W_w']).T], axis=1)
    WB = inp['cnl_theta_w'].T @ w_big
    put(cbB, CBB, 'WB0', WB[:128])
    put(cbB, CBB, 'WB1', WB[128:])
    put(cbB, CBB, 'bb', (inp['cnl_theta_b'] @ w_big)[None, :])

    put(cf, CF, 'w_gT', inp['cnl_g_w'] / Cl)
    bgc = (inp['cnl_g_b'] / Cl)[:, None]
    put(cf, CF, 'b_g', np.concatenate([bgc, bgc], axis=1))
    put(cf, CF, 'b_th2', (inp['pnl_theta_b'] + inp['pnl_theta_w'] @ cnl_bf)[:, None])
    bias2 = (pnl_bf + cnl_bf)
    put(cf, CF, 'b2', np.stack([bias2[:128], bias2[128:]], axis=1))
    fc1 = inp['ca_fc1_w'].T
    put(cf, CF, 'fc1T', np.concatenate([fc1[:128], fc1[128:]], axis=1))
    put(cf, CF, 'fc1TN', np.concatenate([fc1[:128], fc1[128:]], axis=1) / N)
    put(cf, CF, 'fc2T', inp['ca_fc2_w'].T)
    put(cf, CF, 'onef', np.ones((1, 1), dtype=np.float32))

    f['cbA'] = cbA.astype(ml_dtypes.bfloat16)
    f['cbB'] = cbB.astype(ml_dtypes.bfloat16)
    f['cf'] = cf.astype(np.float32)
    return f


def build_nc(w_fuse):
    nc = bacc.Bacc(None)
    x_d = nc.declare_dram_parameter("x", [128, 2, N], BF16, isOutput=False)
    x0_d = nc.declare_dram_parameter("x0", [128, N], BF16, isOutput=False)
    cbA_d = nc.declare_dram_parameter("cbA", [128, CBA_COLS], BF16, isOutput=False)
    cbB_d = nc.declare_dram_parameter("cbB", [128, CBB_COLS], BF16, isOutput=False)
    cf_d = nc.declare_dram_parameter("cf", [128, CF_COLS], F32R, isOutput=False)
    out_d = nc.declare_dram_parameter("out", [256, N], BF16, isOutput=True)

    with tile.TileContext(nc) as tc:
        _frees = []

        def _keep(pair):
            _frees.append(pair[1])
            return pair[0]

        # ---- persistent SBUF tensors ----
        x_t = _keep(tc.tile([128, 2, N], BF16, name="x_t"))
        x0_t = _keep(tc.tile([128, N], BF16, name="x0_t"))
        cbA_t = _keep(tc.tile([128, CBA_COLS], BF16, name="cbA_t"))
        cbB_t = _keep(tc.tile([128, CBB_COLS], BF16, name="cbB_t"))
        cf_t = _keep(tc.tile([128, CF_COLS], F32R, name="cf_t"))
        x0cat = _keep(tc.tile([128, 32, 256], BF16, name="x0cat"))
        fold1_s = _keep(tc.tile([128, 320], F32R, name="fold1_s"))
        WDC_s = _keep(tc.tile([128, 320], BF16, name="WDC_s"))
        S2_s = _keep(tc.tile([128, 128], BF16, name="S2_s"))
        T2 = _keep(tc.tile([128, M], BF16, name="T2"))
        WS_sb = _keep(tc.tile([128, 512], BF16, name="WS_sb"))
        z_t = _keep(tc.tile([128, 2, N], BF16, name="z_t"))
        bz = _keep(tc.tile([128, 2], F32, name="bz"))
        bT2 = _keep(tc.tile([128, 1], F32, name="bT2"))
        psum_cols = _keep(tc.tile([128, 2, 8], F32, name="psum_cols"))
        macc = _keep(tc.tile([128, 2, 512], BF16, name="macc"))
        V_t = _keep(tc.tile([128, 2, 2], F32, name="V_t"))
        h_t = _keep(tc.tile([16, 2], F32, name="h_t"))
        ca_t = _keep(tc.tile([128, 2], F32, name="ca_t"))
        ca_bf = _keep(tc.tile([128, 2], BF16, name="ca_bf"))
        tmp1 = _keep(tc.tile([128, 4], F32, name="tmp1"))
        xp_t = _keep(tc.tile([128, 2, N], BF16, name="xp_t"))
        tA = _keep(tc.tile([128, N], BF16, name="tA"))
        mapT_meanP = _keep(tc.tile([64, 64], BF16, name="mapT_meanP"))
        mapT_maxP = _keep(tc.tile([64, 64], BF16, name="mapT_maxP"))
        R_sb = _keep(tc.tile([64, 448], BF16, name="R_sb"))
        sig2d = _keep(tc.tile([64, 64], BF16, name="sig2d"))
        sigb = _keep(tc.tile([128, 1, N], BF16, name="sigb"))

        def cA(name, rows=None):
            off, cols, rws = CBA[name]
            return cbA_t[0:(rows or rws), off:off + cols]

        def cB(name, rows=None):
            off, cols, rws = CBB[name]
            return cbB_t[0:(rows or rws), off:off + cols]

        def cF(name, rows=None):
            off, cols, rws = CF[name]
            return cf_t[0:(rows or rws), off:off + cols]

        from contextlib import ExitStack
        stack = ExitStack()

        # ---- DMAs: first pixel group + early consts, then the rest ----
        nc.sync.dma_start(out=x0_t[:, 0:512], in_=x0_d[:, 0:512])
        nc.sync.dma_start(out=x_t[:, :, 0:512], in_=x_d[:, :, 0:512])
        nc.sync.dma_start(out=cbA_t[:, :], in_=cbA_d[:, :])
        nc.sync.dma_start(out=x0_t[:, 512:2048], in_=x0_d[:, 512:2048])
        nc.sync.dma_start(out=x_t[:, :, 512:2048], in_=x_d[:, :, 512:2048])
        nc.sync.dma_start(out=x0_t[:, 2048:4096], in_=x0_d[:, 2048:4096])
        nc.sync.dma_start(out=x_t[:, :, 2048:4096], in_=x_d[:, :, 2048:4096])
        nc.sync.dma_start(out=cbB_t[:, :], in_=cbB_d[:, :])
        nc.sync.dma_start(out=cf_t[:, :], in_=cf_d[:, :])

        sp = stack.enter_context(tc.tile_pool(name="sp", bufs=3))

        # warm the sigmoid act-table set (contains identity/copy/relu too)
        warm = sp.tile([1, 8], F32, tag="warm", name="warm", bufs=1)
        nc.vector.memset(warm[:, :], 0.0)
        nc.scalar.activation(out=warm[:, :], in_=warm[:, :], func=AF.Sigmoid)
        onescol = sp.tile([128, 1], BF16, tag="onescol", name="onescol",
                          bufs=1)
        nc.vector.memset(onescol[:, :], 1.0)

        # xp = (1-w)*x on the otherwise-idle Pool engine (SBUF-only there);
        # each chunk is pre-written to out_d in the idle DMA window so the
        # final add happens via DMA accumulate instead of DVE
        for g in range(4):
            nc.gpsimd.tensor_scalar(out=xp_t[:, :, bass.ts(g, 1024)],
                                    in0=x_t[:, :, bass.ts(g, 1024)],
                                    scalar1=1.0 - w_fuse, scalar2=None,
                                    op0=ALU.mult)
            nc.sync.dma_start(
                out=out_d[:, bass.ts(g, 1024)].rearrange(
                    "(two p) n -> p two n", two=2),
                in_=xp_t[:, :, bass.ts(g, 1024)])

        # =========== Stage A: x0cat + G = x@ph^T (theta never applied
        # per-pixel: att = w_th^T G + b_th (x) colsum(ph)) ===========
        ps1_ctx = tc.tile_pool(name="ps1", bufs=1, space="PSUM")
        ps1 = ps1_ctx.__enter__()
        ps_s = ps1.tile([64, 256], F32, tag="S2", name="ps_s")
        with tc.tile_pool(name="psA", bufs=2, space="PSUM") as psA:
            G_ps = psA.tile([128, 2, 128], F32, tag="G", name="G_ps", bufs=1)
            cs_ps = psA.tile([1, 128], F32, tag="cs", name="cs_ps", bufs=1)
            for t8 in range(8):
                ps_x0c = psA.tile([128, 1024], F32, tag="x0c", name="ps_x0c")
                for sub in range(4):
                    i = 4 * t8 + sub
                    nc.tensor.matmul(ps_x0c[:, bass.ts(sub, 256)],
                                     x0_t[:, bass.ts(i, 128)], cA('w_x0cat'),
                                     start=True, stop=False)
                    # fold the S/Y2-part bias in via a rank-1 matmul so its
                    # drain is a plain copy (Pool cannot read PSUM)
                    nc.tensor.matmul(ps_x0c[:, 256 * sub + 128:
                                             256 * sub + 256],
                                     cA('ones1'), cA('b_x0cat', 1)[:, 128:256],
                                     start=False, stop=True)
                pv = ps_x0c[:, :].rearrange("p (a c) -> p a c", c=256)
                bv = cA('b_x0cat').rearrange("p (a c) -> p a c", c=256)
                # urgent (att) part biased on DVE, lazy part copied on Act
                nc.vector.tensor_tensor(
                    out=x0cat[:, 4 * t8:4 * t8 + 4, 0:128],
                    in0=pv[:, :, 0:128],
                    in1=bv[:, :, 0:128].broadcast_to([128, 4, 128]),
                    op=ALU.add)
                nc.scalar.activation(
                    out=x0cat[:, 4 * t8:4 * t8 + 4, 128:256],
                    in_=pv[:, :, 128:256], func=AF.Copy)
                for sub in range(4):
                    i = 4 * t8 + sub
                    st = (i == 0)
                    sp_ = (i == 31)
                    for ch in range(2):
                        nc.tensor.matmul(G_ps[:, ch, :],
                                         x_t[:, ch, bass.ts(i, 128)],
                                         x0cat[:, i, 0:128],
                                         start=st, stop=sp_)
                    nc.tensor.matmul(cs_ps[:, :], onescol[:, :],
                                     x0cat[:, i, 0:128],
                                     start=st, stop=sp_)
            G_sb = sp.tile([128, 2, 128], BF16, tag="G_sb", name="G_sb",
                           bufs=1)
            nc.vector.tensor_copy(out=G_sb[:, 0, :], in_=G_ps[:, 0, :])
            nc.scalar.activation(out=G_sb[:, 1, :], in_=G_ps[:, 1, :],
                                 func=AF.Copy)
            cs_sb = sp.tile([1, 128], BF16, tag="cs_sb", name="cs_sb", bufs=1)
            nc.vector.tensor_copy(out=cs_sb[:, :], in_=cs_ps[:, :])

        # =========== folds + T + z + channel attention ===========
        with tc.tile_pool(name="psB", bufs=2, space="PSUM") as psB:
            # S blocks transposed (stat=G, mov=P): S2T[g, c]; consecutive
            # emission (interleaving the four shared-bank psum streams with
            # other matmuls corrupts the accumulation)
            for j in range(16):
                st = (j == 0)
                sp_ = (j == 15)
                GTa = x0cat[:, j, 192:256]
                GTb = x0cat[:, j + 16, 192:256]
                Ppair = x0cat[:, j:j + 17:16, 128:192]
                nc.tensor.matmul(ps_s[:, 0:128].rearrange(
                                     "p (a b) -> p a b", a=2),
                                 GTa, Ppair, start=st, stop=sp_)
                nc.tensor.matmul(ps_s[:, 128:256].rearrange(
                                     "p (a b) -> p a b", a=2),
                                 GTb, Ppair, start=st, stop=sp_)
            nc.vector.tensor_copy(out=S2_s[0:64, :], in_=ps_s[:, 0:128])
            nc.vector.tensor_copy(out=S2_s[64:128, :], in_=ps_s[:, 128:256])
            # fold1 = G^T WB + cs (x) bb  -> [wta | WA] (att never built)
            ps_f1 = psB.tile([128, 320], F32, tag="sm", name="ps_f1",
                             bufs=1)
            nc.tensor.matmul(ps_f1[:, :], G_sb[:, 0, :], cB('WB0'),
                             start=True, stop=False)
            nc.tensor.matmul(ps_f1[:, :], G_sb[:, 1, :], cB('WB1'),
                             start=False, stop=False)
            nc.tensor.matmul(ps_f1[:, :], cs_sb[:, :], cB('bb', 1),
                             start=False, stop=True)
            nc.scalar.copy(out=fold1_s[:, :], in_=ps_f1[:, :])
            # fold2 = w_gT @ [wta | WA] -> [WD | WC]
            ps_f2 = psB.tile([128, 320], F32, tag="sm", name="ps_f2",
                             bufs=1)
            nc.tensor.matmul(ps_f2[:, :], _R(cF('w_gT')), fold1_s[:, :],
                             start=True, stop=True)
            nc.vector.tensor_copy(out=WDC_s[:, :], in_=ps_f2[:, :])
            # bT2 = wta^T b_g + b_th2 (row-broadcast over partitions)
            ps_bt = psB.tile([64, 2], F32, tag="sm", name="ps_bt", bufs=1)
            nc.tensor.matmul(ps_bt[:, :], fold1_s[:, 0:64], _R(cF('b_g')),
                             start=True, stop=True)
            nc.vector.tensor_tensor(out=bT2[0:64, :], in0=ps_bt[:, 0:1],
                                    in1=cF('b_th2').bitcast(F32), op=ALU.add)
            nc.vector.tensor_copy(out=bT2[64:128, :], in_=bT2[0:64, :])
            # bz = WA^T b_g + b2
            ps_bb = psB.tile([128, 4], F32, tag="sm", name="ps_bb", bufs=1)
            nc.tensor.matmul(ps_bb[:, 0:2], fold1_s[:, 64:192], _R(cF('b_g')),
                             start=True, stop=True)
            nc.tensor.matmul(ps_bb[:, 2:4], fold1_s[:, 192:320], _R(cF('b_g')),
                             start=True, stop=True)
            nc.vector.tensor_tensor(out=bz[:, 0:1], in0=ps_bb[:, 0:1],
                                    in1=cF('b2')[:, 0:1].bitcast(F32), op=ALU.add)
            nc.vector.tensor_tensor(out=bz[:, 1:2], in0=ps_bb[:, 2:3],
                                    in1=cF('b2')[:, 1:2].bitcast(F32), op=ALU.add)

            # WS = S2T-halves contracted with w_pnlW: z reads T2 directly
            ps_ws = psB.tile([128, 512], F32, tag="ws", name="ps_ws", bufs=1)
            for h in range(2):
                for ch in range(2):
                    nc.tensor.matmul(ps_ws[:, bass.ts(2 * h + ch, 128)],
                                     S2_s[64 * h:64 * h + 64, :],
                                     cB('w_pnlW')[64 * h:64 * h + 64,
                                                  bass.ts(ch, 128)],
                                     start=True, stop=True)
            nc.vector.tensor_copy(out=WS_sb[:, :], in_=ps_ws[:, :])

            # ---- T2 [128, M] interleaved with z emission ----
            def emit_T2(tm):
                ps_T = psB.tile([128, 512], F32, tag="TY", name="ps_T")
                for h in range(2):
                    base = h * M + tm * 512
                    o = ps_T[64 * h:64 * h + 64, :]
                    nc.tensor.matmul(o, cB('w_th2')[:, 0:64],
                                     x_t[:, 0, base:base + 512],
                                     start=True, stop=False)
                    nc.tensor.matmul(o, cB('w_th2')[:, 64:128],
                                     x_t[:, 1, base:base + 512],
                                     start=False, stop=False)
                    nc.tensor.matmul(o, WDC_s[:, 0:64], x0_t[:, base:base + 512],
                                     start=False, stop=True)
                nc.scalar.activation(out=T2[:, bass.ts(tm, 512)], in_=ps_T[:, :],
                                     func=AF.Identity, bias=bT2[:, :])

            for tm in range(4):
                emit_T2(tm)

            # ---- z [128, 2, N] bf16 ----
            for t in range(8):
                h = t // 4
                mbase = (t % 4) * 512
                for ch in range(2):
                    ps_z = psB.tile([128, 512], F32, tag="z", name="ps_z",
                                    bufs=3)
                    nc.tensor.matmul(ps_z[:, :],
                                     WS_sb[:, bass.ts(2 * h + ch, 128)],
                                     T2[:, mbase:mbase + 512],
                                     start=True, stop=False)
                    act_path = (ch == 0 and t < 5) or (ch == 1 and t >= 3)
                    nc.tensor.matmul(ps_z[:, :], WDC_s[:, 64 + 128 * ch:
                                                       192 + 128 * ch],
                                     x0_t[:, bass.ts(t, 512)],
                                     start=False, stop=not act_path)
                    if act_path:
                        nc.tensor.matmul(ps_z[:, :], cA('ident_bf'),
                                         x_t[:, ch, bass.ts(t, 512)],
                                         start=False, stop=True)
                        nc.scalar.activation(
                            out=z_t[:, ch, bass.ts(t, 512)], in_=ps_z[:, :],
                            func=AF.Identity, bias=bz[:, ch:ch + 1],
                            accum_out=psum_cols[:, ch, t:t + 1])
                    else:
                        nc.vector.scalar_tensor_tensor(
                            out=z_t[:, ch, bass.ts(t, 512)], in0=ps_z[:, :],
                            scalar=bz[:, ch:ch + 1],
                            in1=x_t[:, ch, bass.ts(t, 512)], op0=ALU.add,
                            op1=ALU.add,
                            accum_out=psum_cols[:, ch, t:t + 1])
                # rolling channel-wise max: first half's pixel-reduce runs
                # during z production so only half remains on the ca spine
                if t == 0:
                    nc.vector.tensor_copy(out=macc[:, :, :],
                                          in_=z_t[:, :, 0:512])
                elif t == 4:
                    nc.vector.reduce_max(out=V_t[:, :, 1:2],
                                         in_=macc[:, :, :],
                                         axis=mybir.AxisListType.X)
                    nc.vector.tensor_copy(out=macc[:, :, :],
                                          in_=z_t[:, :, 2048:2560])
                else:
                    nc.vector.tensor_tensor(
                        out=macc[:, :, :], in0=macc[:, :, :],
                        in1=z_t[:, :, bass.ts(t, 512)], op=ALU.max)


            # ---- CBAM channel attention (compressed chain) ----
            nc.vector.reduce_max(out=V_t[:, :, 0:1], in_=macc[:, :, :],
                                 axis=mybir.AxisListType.X)
            nc.vector.tensor_tensor(out=V_t[:, :, 1:2], in0=V_t[:, :, 0:1],
                                    in1=V_t[:, :, 1:2], op=ALU.max)
            nc.vector.reduce_sum(out=V_t[:, :, 0:1], in_=psum_cols[:, :, :],
                                 axis=mybir.AxisListType.X)
            ps_f1b = psB.tile([16, 2], F32, tag="sm", name="ps_f1b",
                              bufs=1)
            nc.tensor.matmul(ps_f1b[:, 0:1], cF('fc1TN')[:, 0:16].bitcast(F32),
                             V_t[:, 0, 0:1], start=True, stop=False)
            nc.tensor.matmul(ps_f1b[:, 0:1], cF('fc1TN')[:, 16:32].bitcast(F32),
                             V_t[:, 1, 0:1], start=False, stop=True)
            nc.tensor.matmul(ps_f1b[:, 1:2], cF('fc1T')[:, 0:16].bitcast(F32),
                             V_t[:, 0, 1:2], start=True, stop=False)
            nc.tensor.matmul(ps_f1b[:, 1:2], cF('fc1T')[:, 16:32].bitcast(F32),
                             V_t[:, 1, 1:2], start=False, stop=True)
            nc.scalar.activation(out=h_t[:, :], in_=ps_f1b[:, :], func=AF.Relu)
            ps_f2b = psB.tile([128, 2, 2], F32, tag="sm", name="ps_f2b",
                              bufs=1)
            for ch in range(2):
                nc.tensor.matmul(ps_f2b[:, ch, :],
                                 cF('fc2T')[:, bass.ts(ch, 128)].bitcast(F32),
                                 h_t[:, :], start=True, stop=True)
            nc.vector.reduce_sum(out=tmp1[:, 0:2], in_=ps_f2b[:, :, :],
                                 axis=mybir.AxisListType.X)
            nc.scalar.activation(out=ca_t[:, :], in_=tmp1[:, 0:2],
                                 func=AF.Sigmoid)
            nc.vector.tensor_copy(out=ca_bf[:, :], in_=ca_t[:, :])

        ps1_ctx.__exit__(None, None, None)

        # =========== maps + sa conv + final ===========
        from concourse import bass_isa
        with tc.tile_pool(name="psC", bufs=2, space="PSUM") as psC:
            # zs = z*ca spread over Act/DVE/Pool; tA + partition-max chase
            # per 1024-chunk; mean mapT built directly from tiny PE matmuls
            # (stationary = z 64-col block, moving = ca column)
            ps_tm = psC.tile([64, 64], F32, tag="tm", name="ps_tm", bufs=1)
            ps_tx = psC.tile([64, 64], F32, tag="tm", name="ps_tx", bufs=1)
            for g in range(4):
                for y in range(16 * g, 16 * g + 16):
                    nc.tensor.matmul(ps_tm[:, y:y + 1],
                                     z_t[:, 0, 64 * y:64 * y + 64],
                                     ca_bf[:, 0:1], start=True, stop=False)
                    nc.tensor.matmul(ps_tm[:, y:y + 1],
                                     z_t[:, 1, 64 * y:64 * y + 64],
                                     ca_bf[:, 1:2], start=False, stop=True)
                for ch in range(2):
                    nc.vector.tensor_scalar(
                        out=z_t[:, ch, bass.ts(g, 1024)],
                        in0=z_t[:, ch, bass.ts(g, 1024)],
                        scalar1=ca_t[:, ch:ch + 1], scalar2=None,
                        op0=ALU.mult)
                # tA = max over the channel pairs; partition-max on Pool
                nc.vector.tensor_tensor(out=tA[:, bass.ts(g, 1024)],
                                        in0=z_t[:, 0, bass.ts(g, 1024)],
                                        in1=z_t[:, 1, bass.ts(g, 1024)],
                                        op=ALU.max)
                mxf = sp.tile([128, 1024], F32, tag="mxf", name="mxf", bufs=2)
                nc.gpsimd.partition_all_reduce(mxf[:, :],
                                               tA[:, bass.ts(g, 1024)], 128,
                                               bass_isa.ReduceOp.max)
                for y in range(16 * g, 16 * g + 16):
                    nc.tensor.transpose(
                        ps_tx[:, y:y + 1],
                        mxf[0:1, 64 * (y - 16 * g):64 * (y - 16 * g) + 64],
                        cF('onef').bitcast(F32))
            nc.vector.tensor_copy(out=mapT_meanP[:, :], in_=ps_tm[:, :])
            nc.vector.tensor_copy(out=mapT_maxP[:, 0:32], in_=ps_tx[:, 0:32])
            nc.vector.tensor_copy(out=mapT_maxP[:, 32:64],
                                  in_=ps_tx[:, 32:64])

            # sa conv (banded) + sigmoid; max stationary split per半 so the
            # first half accumulates while later preduce chunks still run
            ps_R = psC.tile([64, 448], F32, tag="sm2", name="ps_R")
            nc.tensor.matmul(ps_R[:, :], mapT_meanP[:, :], cB('Kcat2')[:, 0:448],
                             start=True, stop=False)
            nc.tensor.matmul(ps_R[0:32, :], mapT_maxP[:, 0:32],
                             cB('Kcat2')[:, 448:896], start=False, stop=True)
            nc.tensor.matmul(ps_R[32:64, :], mapT_maxP[:, 32:64],
                             cB('Kcat2')[:, 448:896], start=False, stop=True)
            nc.vector.tensor_copy(out=R_sb[:, :], in_=ps_R[:, :])
            ps_sa = psC.tile([64, 64], F32, tag="sm2", name="ps_sa")
            for dy in range(7):
                nc.tensor.matmul(ps_sa[:, :], cB('identS')[:, dy:dy + 64],
                                 R_sb[:, bass.ts(dy, 64)],
                                 start=(dy == 0), stop=(dy == 6))
            nc.scalar.activation(out=sig2d[:, :], in_=ps_sa[:, :], func=AF.Sigmoid)

            # sigb broadcast straight from sig2d: stationary is a broadcast
            # w_fuse*ident column (selects row y), moving is the whole map
            # out = zs*sigb + xp, per-group pipelined with DMA out
            for t in range(8):
                ps_bc = psC.tile([128, 512], F32, tag="bc", name="ps_bc")
                for yl in range(8):
                    y = 8 * t + yl
                    nc.tensor.matmul(ps_bc[:, bass.ts(yl, 64)],
                                     cB('wident')[:, y:y + 1].broadcast_to(
                                         [64, 128]),
                                     sig2d[:, :], start=True, stop=True)
                nc.scalar.activation(out=sigb[:, 0, bass.ts(t, 512)],
                                     in_=ps_bc[:, :], func=AF.Copy)
                sl = bass.ts(t, 512)
                if t % 2 == 0:
                    vt2 = sp.tile([128, 2, 1024], BF16, tag="vt2",
                                  name="vt2", bufs=4)
                sgb = sigb[:, :, sl].broadcast_to([128, 2, 512])
                nc.vector.tensor_tensor(
                    out=vt2[:, :, 512 * (t % 2):512 * (t % 2) + 512],
                    in0=z_t[:, :, sl], in1=sgb, op=ALU.mult)
                if t % 2 == 1:
                    g = t // 2
                    nc.gpsimd.dma_start(
                        out=out_d[:, bass.ts(g, 1024)].rearrange(
                            "(two p) n -> p two n", two=2),
                        in_=vt2[:, :, :], accum_op=ALU.add)
        stack.close()
        for fr in reversed(_frees):
            fr()
    nc.compile()
    return nc


_CACHE = {}


def kernel(**inputs):
    inp = {k: np.asarray(v) for k, v in inputs.items()}
    f = fold_params(inp)
    key = round(f['w_fuse'], 9)
    if key not in _CACHE:
        _CACHE[key] = build_nc(f['w_fuse'])
    nc = _CACHE[key]

    B = inp['x'].shape[0]
    in_maps = []
    for b in range(B):
        xb = inp['x'][b].reshape(256, N).astype(np.float32)
        m = {
            'x': np.ascontiguousarray(
                xb.reshape(2, 128, N).transpose(1, 0, 2)).astype(ml_dtypes.bfloat16),
            'x0': np.ascontiguousarray(
                inp['x0'][b].reshape(128, N)).astype(ml_dtypes.bfloat16),
            'cbA': f['cbA'], 'cbB': f['cbB'], 'cf': f['cf'],
        }
        in_maps.append(m)

    res = run_bass_kernel_spmd(nc, in_maps, core_ids=list(range(B)))
    out = np.stack([np.asarray(res.results[b]['out'], dtype=np.float32
                               ).reshape(256, H, W) for b in range(B)])
    return out


# revision 61
# speedup vs baseline: 1.0132x; 1.0029x over previous
"""Trainium2 Bass kernel for nn_MDFO (CNL + PNL non-local blocks + CBAM + fusion).

Restructured v7 (pure data-parallel, B=8 over 8 cores, params replicated):
  - bf16 inputs (x, x0) uploaded from host; bf16 output, fp32 on host.
  - all constants packed into three blob DMAs (early-bf16, late-bf16, f32).
  - theta never applied per-pixel and att never materialized: stage A only
    accumulates G = x @ ph^T and colsum(ph); fold1 = G^T WB + cs (x) bb with
    WB = w_th @ [w_tyT|w_cnlW] folded host-side; fold2 = w_gT @ fold1.
  - Y2 never materialized: WS = S2T-halves @ w_pnlW lets z contract T2
    directly; S blocks emitted transposed with paired G columns.
  - z emission split across Act (ident-matmul + bias path) and DVE stt,
    with the CBAM mean accumulated via accum_out and the channel max via
    two rolling chains (first half's pixel reduce runs during z).
  - mean mapT built by tiny per-column matmuls (stationary = z 64-col
    block, moving = ca column); max map via Pool partition_all_reduce into
    replicated f32 rows + tiny f32 column transposes; Sdy has no perm.
  - sigmoid broadcast straight from sig2d via broadcast-stationary
    w*ident-column matmuls (no sigrow extraction).
  - (1-w)*x computed on Pool and pre-written to out_d during the idle DMA
    window; the final z*ca*sigb multiply is accumulated on top with
    software-DGE accum DMAs, eliminating the final add pass entirely.
"""
import sys

import numpy as np

sys.path.insert(0, "/opt/trn_rl_repo")

import ml_dtypes  # noqa: E402

import concourse.bass as bass  # noqa: E402
import concourse.bacc as bacc  # noqa: E402
import concourse.tile as tile  # noqa: E402
from concourse import mybir  # noqa: E402
from concourse.bass_utils import run_bass_kernel_spmd  # noqa: E402

EPS = 1e-5
F32 = mybir.dt.float32
F32R = mybir.dt.float32r
BF16 = mybir.dt.bfloat16
AF = mybir.ActivationFunctionType
ALU = mybir.AluOpType

Ch, Cl, H, W = 256, 128, 64, 64
N = H * W            # 4096
M = N // 2           # 2048
r = Cl // 2          # 64

# blob layouts: name -> (col offset, cols, rows)
CBA_COLS = 768   # early bf16 blob
CBA = {'w_x0cat': (0, 256, 128), 'b_x0cat': (256, 256, 128),
       'ones1': (512, 128, 1), 'ident_bf': (640, 128, 128)}
CBB_COLS = 2374  # late bf16 blob
CBB = {'w_th2': (0, 128, 128), 'w_pnlW': (128, 256, 128),
       'Kcat2': (384, 896, 64), 'identS': (1280, 70, 64),
       'wident': (1350, 64, 64), 'WB0': (1414, 320, 128),
       'WB1': (1734, 320, 128), 'bb': (2054, 320, 1)}
CF_COLS = 454    # f32 blob
CF = {'w_gT': (0, 128, 128), 'b_g': (128, 2, 128),
      'b_th2': (130, 1, 64), 'b2': (131, 2, 128), 'fc1T': (133, 32, 128),
      'fc2T': (165, 256, 16), 'onef': (421, 1, 1), 'fc1TN': (422, 32, 128)}


def _R(ap):
    return ap.bitcast(F32R)


def fold_params(inp):
    """Host-side constant folding into three blob arrays."""
    f = {}
    scale1 = inp['cnl_bn_g'] / np.sqrt(inp['cnl_bn_v'] + EPS)
    cnl_bf = (inp['cnl_W_b'] * scale1 + inp['cnl_bn_b']
              - inp['cnl_bn_m'] * scale1).astype(np.float32)
    scale2 = inp['pnl_bn_g'] / np.sqrt(inp['pnl_bn_v'] + EPS)
    pnl_bf = (inp['pnl_W_b'] * scale2 + inp['pnl_bn_b']
              - inp['pnl_bn_m'] * scale2).astype(np.float32)
    w_fuse = float(inp['fusion_weight'])
    f['w_fuse'] = w_fuse

    cbA = np.zeros((128, CBA_COLS), dtype=np.float32)
    cbB = np.zeros((128, CBB_COLS), dtype=np.float32)
    cf = np.zeros((128, CF_COLS), dtype=np.float32)

    def put(blob, table, name, arr):
        off, cols, rows = table[name]
        blob[:rows, off:off + cols] = arr

    put(cbA, CBA, 'w_x0cat', np.concatenate([
        inp['cnl_phi_w'].T, inp['pnl_phi_w'].T, (inp['pnl_g_w'] / M).T],
        axis=1))
    brow = np.concatenate([inp['cnl_phi_b'], inp['pnl_phi_b'],
                           inp['pnl_g_b'] / M])
    put(cbA, CBA, 'b_x0cat', np.tile(brow[None, :], (128, 1)))
    put(cbA, CBA, 'ones1', np.ones((1, 128), dtype=np.float32))
    put(cbA, CBA, 'ident_bf', np.eye(128, dtype=np.float32))

    th2 = inp['pnl_theta_w'].T
    put(cbB, CBB, 'w_th2', np.concatenate([th2[:128], th2[128:]], axis=1))
    w_pnlW = (scale2[:, None] * inp['pnl_W_w']).T
    put(cbB, CBB, 'w_pnlW', np.concatenate([w_pnlW, w_pnlW], axis=0))
    # sa conv banded mats; only 1/256 fold on the mean channel (no w folds)
    sa_w = np.asarray(inp['sa_conv_w'][0], dtype=np.float32).copy()
    sa_w[0] /= 256.0
    Kcat = np.zeros((2, 64, 7 * 64), dtype=np.float32)
    for ch in range(2):
        for dy in range(7):
            for dx in range(7):
                w_ = sa_w[ch, dy, dx]
                if w_ == 0.0:
                    continue
                for x in range(64):
                    xq = x + dx - 3
                    if 0 <= xq < 64:
                        Kcat[ch, xq, dy * 64 + x] = w_
    put(cbB, CBB, 'Kcat2', np.concatenate([Kcat[0], Kcat[1]], axis=1))
    # shifted identity: identS[p, c] = 1 iff p == c-3; slices [dy:dy+64]
    # reproduce the Sdy one-hot selector blocks of the banded sa conv
    identS = np.zeros((64, 70), dtype=np.float32)
    for c in range(3, 67):
        identS[c - 3, c] = 1.0
    put(cbB, CBB, 'identS', identS)
    put(cbB, CBB, 'wident', w_fuse * np.eye(64, dtype=np.float32))
    # fold theta through the fold-1 weights: fold1 = G^T WB + cs (x) bb
    w_big = np.concatenate([
        (inp['pnl_theta_w'] @ (scale1[:, None] * inp['cnl_W_w'])).T,
        (scale1[:, None] * inp['cnl_W_w']).T], axis=1)
    WB = inp['cnl_theta_w'].T @ w_big
    put(cbB, CBB, 'WB0', WB[:128])
    put(cbB, CBB, 'WB1', WB[128:])
    put(cbB, CBB, 'bb', (inp['cnl_theta_b'] @ w_big)[None, :])

    put(cf, CF, 'w_gT', inp['cnl_g_w'] / Cl)
    bgc = (inp['cnl_g_b'] / Cl)[:, None]
    put(cf, CF, 'b_g', np.concatenate([bgc, bgc], axis=1))
    put(cf, CF, 'b_th2', (inp['pnl_theta_b'] + inp['pnl_theta_w'] @ cnl_bf)[:, None])
    bias2 = (pnl_bf + cnl_bf)
    put(cf, CF, 'b2', np.stack([bias2[:128], bias2[128:]], axis=1))
    fc1 = inp['ca_fc1_w'].T
    put(cf, CF, 'fc1T', np.concatenate([fc1[:128], fc1[128:]], axis=1))
    put(cf, CF, 'fc1TN', np.concatenate([fc1[:128], fc1[128:]], axis=1) / N)
    put(cf, CF, 'fc2T', inp['ca_fc2_w'].T)
    put(cf, CF, 'onef', np.ones((1, 1), dtype=np.float32))

    f['cbA'] = cbA.astype(ml_dtypes.bfloat16)
    f['cbB'] = cbB.astype(ml_dtypes.bfloat16)
    f['cf'] = cf.astype(np.float32)
    return f


def build_nc(w_fuse):
    nc = bacc.Bacc(None)
    x_d = nc.declare_dram_parameter("x", [128, 2, N], BF16, isOutput=False)
    x0_d = nc.declare_dram_parameter("x0", [128, N], BF16, isOutput=False)
    cbA_d = nc.declare_dram_parameter("cbA", [128, CBA_COLS], BF16, isOutput=False)
    cbB_d = nc.declare_dram_parameter("cbB", [128, CBB_COLS], BF16, isOutput=False)
    cf_d = nc.declare_dram_parameter("cf", [128, CF_COLS], F32R, isOutput=False)
    out_d = nc.declare_dram_parameter("out", [256, N], BF16, isOutput=True)

    with tile.TileContext(nc) as tc:
        _frees = []

        def _keep(pair):
            _frees.append(pair[1])
            return pair[0]

        # ---- persistent SBUF tensors ----
        x_t = _keep(tc.tile([128, 2, N], BF16, name="x_t"))
        x0_t = _keep(tc.tile([128, N], BF16, name="x0_t"))
        cbA_t = _keep(tc.tile([128, CBA_COLS], BF16, name="cbA_t"))
        cbB_t = _keep(tc.tile([128, CBB_COLS], BF16, name="cbB_t"))
        cf_t = _keep(tc.tile([128, CF_COLS], F32R, name="cf_t"))
        x0cat = _keep(tc.tile([128, 32, 256], BF16, name="x0cat"))
        fold1_s = _keep(tc.tile([128, 320], F32R, name="fold1_s"))
        WDC_s = _keep(tc.tile([128, 320], BF16, name="WDC_s"))
        S2_s = _keep(tc.tile([128, 128], BF16, name="S2_s"))
        T2 = _keep(tc.tile([128, M], BF16, name="T2"))
        WS_sb = _keep(tc.tile([128, 512], BF16, name="WS_sb"))
        z_t = _keep(tc.tile([128, 2, N], BF16, name="z_t"))
        bz = _keep(tc.tile([128, 2], F32, name="bz"))
        bT2 = _keep(tc.tile([128, 1], F32, name="bT2"))
        psum_cols = _keep(tc.tile([128, 2, 8], F32, name="psum_cols"))
        macc = _keep(tc.tile([128, 2, 512], BF16, name="macc"))
        V_t = _keep(tc.tile([128, 2, 2], F32, name="V_t"))
        h_t = _keep(tc.tile([16, 2], F32, name="h_t"))
        ca_t = _keep(tc.tile([128, 2], F32, name="ca_t"))
        ca_bf = _keep(tc.tile([128, 2], BF16, name="ca_bf"))
        tmp1 = _keep(tc.tile([128, 4], F32, name="tmp1"))
        xp_t = _keep(tc.tile([128, 2, N], BF16, name="xp_t"))
        tA = _keep(tc.tile([128, N], BF16, name="tA"))
        mapT_meanP = _keep(tc.tile([64, 64], BF16, name="mapT_meanP"))
        mapT_maxP = _keep(tc.tile([64, 64], BF16, name="mapT_maxP"))
        R_sb = _keep(tc.tile([64, 448], BF16, name="R_sb"))
        sig2d = _keep(tc.tile([64, 64], BF16, name="sig2d"))
        sigb = _keep(tc.tile([128, 1, N], BF16, name="sigb"))

        def cA(name, rows=None):
            off, cols, rws = CBA[name]
            return cbA_t[0:(rows or rws), off:off + cols]

        def cB(name, rows=None):
            off, cols, rws = CBB[name]
            return cbB_t[0:(rows or rws), off:off + cols]

        def cF(name, rows=None):
            off, cols, rws = CF[name]
            return cf_t[0:(rows or rws), off:off + cols]

        from contextlib import ExitStack
        stack = ExitStack()

        # ---- DMAs: first pixel group + early consts, then the rest ----
        nc.sync.dma_start(out=x0_t[:, 0:512], in_=x0_d[:, 0:512])
        nc.sync.dma_start(out=x_t[:, :, 0:512], in_=x_d[:, :, 0:512])
        nc.sync.dma_start(out=cbA_t[:, :], in_=cbA_d[:, :])
        nc.sync.dma_start(out=x0_t[:, 512:2048], in_=x0_d[:, 512:2048])
        nc.sync.dma_start(out=x_t[:, :, 512:2048], in_=x_d[:, :, 512:2048])
        nc.sync.dma_start(out=x0_t[:, 2048:4096], in_=x0_d[:, 2048:4096])
        nc.sync.dma_start(out=x_t[:, :, 2048:4096], in_=x_d[:, :, 2048:4096])
        nc.sync.dma_start(out=cbB_t[:, :], in_=cbB_d[:, :])
        nc.sync.dma_start(out=cf_t[:, :], in_=cf_d[:, :])

        sp = stack.enter_context(tc.tile_pool(name="sp", bufs=3))

        # warm the sigmoid act-table set (contains identity/copy/relu too)
        warm = sp.tile([1, 8], F32, tag="warm", name="warm", bufs=1)
        nc.vector.memset(warm[:, :], 0.0)
        nc.scalar.activation(out=warm[:, :], in_=warm[:, :], func=AF.Sigmoid)
        onescol = sp.tile([128, 1], BF16, tag="onescol", name="onescol",
                          bufs=1)
        nc.vector.memset(onescol[:, :], 1.0)

        # xp = (1-w)*x on the otherwise-idle Pool engine (SBUF-only there);
        # each chunk is pre-written to out_d in the idle DMA window so the
        # final add happens via DMA accumulate instead of DVE
        for g in range(4):
            nc.gpsimd.tensor_scalar(out=xp_t[:, :, bass.ts(g, 1024)],
                                    in0=x_t[:, :, bass.ts(g, 1024)],
                                    scalar1=1.0 - w_fuse, scalar2=None,
                                    op0=ALU.mult)
            nc.sync.dma_start(
                out=out_d[:, bass.ts(g, 1024)].rearrange(
                    "(two p) n -> p two n", two=2),
                in_=xp_t[:, :, bass.ts(g, 1024)])

        # =========== Stage A: x0cat + G = x@ph^T (theta never applied
        # per-pixel: att = w_th^T G + b_th (x) colsum(ph)) ===========
        ps1_ctx = tc.tile_pool(name="ps1", bufs=1, space="PSUM")
        ps1 = ps1_ctx.__enter__()
        ps_s = ps1.tile([64, 256], F32, tag="S2", name="ps_s")
        with tc.tile_pool(name="psA", bufs=2, space="PSUM") as psA:
            G_ps = psA.tile([128, 2, 128], F32, tag="G", name="G_ps", bufs=1)
            cs_ps = psA.tile([1, 128], F32, tag="cs", name="cs_ps", bufs=1)
            for t8 in range(8):
                ps_x0c = psA.tile([128, 1024], F32, tag="x0c", name="ps_x0c")
                for sub in range(4):
                    i = 4 * t8 + sub
                    nc.tensor.matmul(ps_x0c[:, bass.ts(sub, 256)],
                                     x0_t[:, bass.ts(i, 128)], cA('w_x0cat'),
                                     start=True, stop=False)
                    # fold the S/Y2-part bias in via a rank-1 matmul so its
                    # drain is a plain copy (Pool cannot read PSUM)
                    nc.tensor.matmul(ps_x0c[:, 256 * sub + 128:
                                             256 * sub + 256],
                                     cA('ones1'), cA('b_x0cat', 1)[:, 128:256],
                                     start=False, stop=True)
                pv = ps_x0c[:, :].rearrange("p (a c) -> p a c", c=256)
                bv = cA('b_x0cat').rearrange("p (a c) -> p a c", c=256)
                # urgent (att) part biased on DVE, lazy part copied on Act
                nc.vector.tensor_tensor(
                    out=x0cat[:, 4 * t8:4 * t8 + 4, 0:128],
                    in0=pv[:, :, 0:128],
                    in1=bv[:, :, 0:128].broadcast_to([128, 4, 128]),
                    op=ALU.add)
                nc.scalar.activation(
                    out=x0cat[:, 4 * t8:4 * t8 + 4, 128:256],
                    in_=pv[:, :, 128:256], func=AF.Copy)
                for sub in range(4):
                    i = 4 * t8 + sub
                    st = (i == 0)
                    sp_ = (i == 31)
                    for ch in range(2):
                        nc.tensor.matmul(G_ps[:, ch, :],
                                         x_t[:, ch, bass.ts(i, 128)],
                                         x0cat[:, i, 0:128],
                                         start=st, stop=sp_)
                    nc.tensor.matmul(cs_ps[:, :], onescol[:, :],
                                     x0cat[:, i, 0:128],
                                     start=st, stop=sp_)
            G_sb = sp.tile([128, 2, 128], BF16, tag="G_sb", name="G_sb",
                           bufs=1)
            nc.vector.tensor_copy(out=G_sb[:, 0, :], in_=G_ps[:, 0, :])
            nc.scalar.activation(out=G_sb[:, 1, :], in_=G_ps[:, 1, :],
                                 func=AF.Copy)
            cs_sb = sp.tile([1, 128], BF16, tag="cs_sb", name="cs_sb", bufs=1)
            nc.vector.tensor_copy(out=cs_sb[:, :], in_=cs_ps[:, :])

        # =========== folds + T + z + channel attention ===========
        with tc.tile_pool(name="psB", bufs=2, space="PSUM") as psB:
            # S blocks transposed (stat=G, mov=P): S2T[g, c]; consecutive
            # emission (interleaving the four shared-bank psum streams with
            # other matmuls corrupts the accumulation)
            for j in range(16):
                st = (j == 0)
                sp_ = (j == 15)
                GTa = x0cat[:, j, 192:256]
                GTb = x0cat[:, j + 16, 192:256]
                Ppair = x0cat[:, j:j + 17:16, 128:192]
                nc.tensor.matmul(ps_s[:, 0:128].rearrange(
                                     "p (a b) -> p a b", a=2),
                                 GTa, Ppair, start=st, stop=sp_)
                nc.tensor.matmul(ps_s[:, 128:256].rearrange(
                                     "p (a b) -> p a b", a=2),
                                 GTb, Ppair, start=st, stop=sp_)
            nc.vector.tensor_copy(out=S2_s[0:64, :], in_=ps_s[:, 0:128])
            nc.vector.tensor_copy(out=S2_s[64:128, :], in_=ps_s[:, 128:256])
            # fold1 = G^T WB + cs (x) bb  -> [wta | WA] (att never built)
            ps_f1 = psB.tile([128, 320], F32, tag="sm", name="ps_f1",
                             bufs=1)
            nc.tensor.matmul(ps_f1[:, :], G_sb[:, 0, :], cB('WB0'),
                             start=True, stop=False)
            nc.tensor.matmul(ps_f1[:, :], G_sb[:, 1, :], cB('WB1'),
                             start=False, stop=False)
            nc.tensor.matmul(ps_f1[:, :], cs_sb[:, :], cB('bb', 1),
                             start=False, stop=True)
            nc.scalar.copy(out=fold1_s[:, :], in_=ps_f1[:, :])
            # fold2 = w_gT @ [wta | WA] -> [WD | WC]
            ps_f2 = psB.tile([128, 320], F32, tag="sm", name="ps_f2",
                             bufs=1)
            nc.tensor.matmul(ps_f2[:, :], _R(cF('w_gT')), fold1_s[:, :],
                             start=True, stop=True)
            nc.vector.tensor_copy(out=WDC_s[:, :], in_=ps_f2[:, :])
            # bT2 = wta^T b_g + b_th2 (row-broadcast over partitions)
            ps_bt = psB.tile([64, 2], F32, tag="sm", name="ps_bt", bufs=1)
            nc.tensor.matmul(ps_bt[:, :], fold1_s[:, 0:64], _R(cF('b_g')),
                             start=True, stop=True)
            nc.vector.tensor_tensor(out=bT2[0:64, :], in0=ps_bt[:, 0:1],
                                    in1=cF('b_th2').bitcast(F32), op=ALU.add)
            nc.vector.tensor_copy(out=bT2[64:128, :], in_=bT2[0:64, :])
            # bz = WA^T b_g + b2
            ps_bb = psB.tile([128, 4], F32, tag="sm", name="ps_bb", bufs=1)
            nc.tensor.matmul(ps_bb[:, 0:2], fold1_s[:, 64:192], _R(cF('b_g')),
                             start=True, stop=True)
            nc.tensor.matmul(ps_bb[:, 2:4], fold1_s[:, 192:320], _R(cF('b_g')),
                             start=True, stop=True)
            nc.vector.tensor_tensor(out=bz[:, 0:1], in0=ps_bb[:, 0:1],
                                    in1=cF('b2')[:, 0:1].bitcast(F32), op=ALU.add)
            nc.vector.tensor_tensor(out=bz[:, 1:2], in0=ps_bb[:, 2:3],
                                    in1=cF('b2')[:, 1:2].bitcast(F32), op=ALU.add)

            # WS = S2T-halves contracted with w_pnlW: z reads T2 directly
            ps_ws = psB.tile([128, 512], F32, tag="ws", name="ps_ws", bufs=1)
            for h in range(2):
                for ch in range(2):
                    nc.tensor.matmul(ps_ws[:, bass.ts(2 * h + ch, 128)],
                                     S2_s[64 * h:64 * h + 64, :],
                                     cB('w_pnlW')[64 * h:64 * h + 64,
                                                  bass.ts(ch, 128)],
                                     start=True, stop=True)
            nc.vector.tensor_copy(out=WS_sb[:, :], in_=ps_ws[:, :])

            # ---- T2 [128, M] interleaved with z emission ----
            def emit_T2(tm):
                ps_T = psB.tile([128, 512], F32, tag="TY", name="ps_T")
                for h in range(2):
                    base = h * M + tm * 512
                    o = ps_T[64 * h:64 * h + 64, :]
                    nc.tensor.matmul(o, cB('w_th2')[:, 0:64],
                                     x_t[:, 0, base:base + 512],
                                     start=True, stop=False)
                    nc.tensor.matmul(o, cB('w_th2')[:, 64:128],
                                     x_t[:, 1, base:base + 512],
                                     start=False, stop=False)
                    nc.tensor.matmul(o, WDC_s[:, 0:64], x0_t[:, base:base + 512],
                                     start=False, stop=True)
                nc.scalar.activation(out=T2[:, bass.ts(tm, 512)], in_=ps_T[:, :],
                                     func=AF.Identity, bias=bT2[:, :])

            for tm in range(4):
                emit_T2(tm)

            # ---- z [128, 2, N] bf16 ----
            for t in range(8):
                h = t // 4
                mbase = (t % 4) * 512
                for ch in range(2):
                    ps_z = psB.tile([128, 512], F32, tag="z", name="ps_z",
                                    bufs=3)
                    nc.tensor.matmul(ps_z[:, :],
                                     WS_sb[:, bass.ts(2 * h + ch, 128)],
                                     T2[:, mbase:mbase + 512],
                                     start=True, stop=False)
                    act_path = (ch == 0 and t < 7) or (ch == 1 and t >= 5)
                    nc.tensor.matmul(ps_z[:, :], WDC_s[:, 64 + 128 * ch:
                                                       192 + 128 * ch],
                                     x0_t[:, bass.ts(t, 512)],
                                     start=False, stop=not act_path)
                    if act_path:
                        nc.tensor.matmul(ps_z[:, :], cA('ident_bf'),
                                         x_t[:, ch, bass.ts(t, 512)],
                                         start=False, stop=True)
                        nc.scalar.activation(
                            out=z_t[:, ch, bass.ts(t, 512)], in_=ps_z[:, :],
                            func=AF.Identity, bias=bz[:, ch:ch + 1],
                            accum_out=psum_cols[:, ch, t:t + 1])
                    else:
                        nc.vector.scalar_tensor_tensor(
                            out=z_t[:, ch, bass.ts(t, 512)], in0=ps_z[:, :],
                            scalar=bz[:, ch:ch + 1],
                            in1=x_t[:, ch, bass.ts(t, 512)], op0=ALU.add,
                            op1=ALU.add,
                            accum_out=psum_cols[:, ch, t:t + 1])
                # rolling channel-wise max: first half's pixel-reduce runs
                # during z production so only half remains on the ca spine
                if t == 0:
                    nc.vector.tensor_copy(out=macc[:, :, :],
                                          in_=z_t[:, :, 0:512])
                elif t == 4:
                    nc.vector.reduce_max(out=V_t[:, :, 1:2],
                                         in_=macc[:, :, :],
                                         axis=mybir.AxisListType.X)
                    nc.vector.tensor_copy(out=macc[:, :, :],
                                          in_=z_t[:, :, 2048:2560])
                else:
                    nc.vector.tensor_tensor(
                        out=macc[:, :, :], in0=macc[:, :, :],
                        in1=z_t[:, :, bass.ts(t, 512)], op=ALU.max)


            # ---- CBAM channel attention (compressed chain) ----
            nc.vector.reduce_max(out=V_t[:, :, 0:1], in_=macc[:, :, :],
                                 axis=mybir.AxisListType.X)
            nc.vector.tensor_tensor(out=V_t[:, :, 1:2], in0=V_t[:, :, 0:1],
                                    in1=V_t[:, :, 1:2], op=ALU.max)
            nc.vector.reduce_sum(out=V_t[:, :, 0:1], in_=psum_cols[:, :, :],
                                 axis=mybir.AxisListType.X)
            ps_f1b = psB.tile([16, 2], F32, tag="sm", name="ps_f1b",
                              bufs=1)
            nc.tensor.matmul(ps_f1b[:, 0:1], cF('fc1TN')[:, 0:16].bitcast(F32),
                             V_t[:, 0, 0:1], start=True, stop=False)
            nc.tensor.matmul(ps_f1b[:, 0:1], cF('fc1TN')[:, 16:32].bitcast(F32),
                             V_t[:, 1, 0:1], start=False, stop=True)
            nc.tensor.matmul(ps_f1b[:, 1:2], cF('fc1T')[:, 0:16].bitcast(F32),
                             V_t[:, 0, 1:2], start=True, stop=False)
            nc.tensor.matmul(ps_f1b[:, 1:2], cF('fc1T')[:, 16:32].bitcast(F32),
                             V_t[:, 1, 1:2], start=False, stop=True)
            nc.scalar.activation(out=h_t[:, :], in_=ps_f1b[:, :], func=AF.Relu)
            ps_f2b = psB.tile([128, 2, 2], F32, tag="sm", name="ps_f2b",
                              bufs=1)
            for ch in range(2):
                nc.tensor.matmul(ps_f2b[:, ch, :],
                                 cF('fc2T')[:, bass.ts(ch, 128)].bitcast(F32),
                                 h_t[:, :], start=True, stop=True)
            nc.vector.reduce_sum(out=tmp1[:, 0:2], in_=ps_f2b[:, :, :],
                                 axis=mybir.AxisListType.X)
            nc.scalar.activation(out=ca_t[:, :], in_=tmp1[:, 0:2],
                                 func=AF.Sigmoid)
            nc.vector.tensor_copy(out=ca_bf[:, :], in_=ca_t[:, :])

        ps1_ctx.__exit__(None, None, None)

        # =========== maps + sa conv + final ===========
        from concourse import bass_isa
        with tc.tile_pool(name="psC", bufs=2, space="PSUM") as psC:
            # zs = z*ca spread over Act/DVE/Pool; tA + partition-max chase
            # per 1024-chunk; mean mapT built directly from tiny PE matmuls
            # (stationary = z 64-col block, moving = ca column)
            ps_tm = psC.tile([64, 64], F32, tag="tm", name="ps_tm", bufs=1)
            ps_tx = psC.tile([64, 64], F32, tag="tm", name="ps_tx", bufs=1)
            for g in range(4):
                for y in range(16 * g, 16 * g + 16):
                    nc.tensor.matmul(ps_tm[:, y:y + 1],
                                     z_t[:, 0, 64 * y:64 * y + 64],
                                     ca_bf[:, 0:1], start=True, stop=False)
                    nc.tensor.matmul(ps_tm[:, y:y + 1],
                                     z_t[:, 1, 64 * y:64 * y + 64],
                                     ca_bf[:, 1:2], start=False, stop=True)
                for ch in range(2):
                    nc.vector.tensor_scalar(
                        out=z_t[:, ch, bass.ts(g, 1024)],
                        in0=z_t[:, ch, bass.ts(g, 1024)],
                        scalar1=ca_t[:, ch:ch + 1], scalar2=None,
                        op0=ALU.mult)
                # tA = max over the channel pairs; partition-max on Pool
                nc.vector.tensor_tensor(out=tA[:, bass.ts(g, 1024)],
                                        in0=z_t[:, 0, bass.ts(g, 1024)],
                                        in1=z_t[:, 1, bass.ts(g, 1024)],
                                        op=ALU.max)
                mxf = sp.tile([128, 1024], F32, tag="mxf", name="mxf", bufs=2)
                nc.gpsimd.partition_all_reduce(mxf[:, :],
                                               tA[:, bass.ts(g, 1024)], 128,
                                               bass_isa.ReduceOp.max)
                for y in range(16 * g, 16 * g + 16):
                    nc.tensor.transpose(
                        ps_tx[:, y:y + 1],
                        mxf[0:1, 64 * (y - 16 * g):64 * (y - 16 * g) + 64],
                        cF('onef').bitcast(F32))
            nc.vector.tensor_copy(out=mapT_meanP[:, :], in_=ps_tm[:, :])
            nc.vector.tensor_copy(out=mapT_maxP[:, 0:32], in_=ps_tx[:, 0:32])
            nc.vector.tensor_copy(out=mapT_maxP[:, 32:64],
                                  in_=ps_tx[:, 32:64])

            # sa conv (banded) + sigmoid; max stationary split per半 so the
            # first half accumulates while later preduce chunks still run
            ps_R = psC.tile([64, 448], F32, tag="sm2", name="ps_R")
            nc.tensor.matmul(ps_R[:, :], mapT_meanP[:, :], cB('Kcat2')[:, 0:448],
                             start=True, stop=False)
            nc.tensor.matmul(ps_R[0:32, :], mapT_maxP[:, 0:32],
                             cB('Kcat2')[:, 448:896], start=False, stop=True)
            nc.tensor.matmul(ps_R[32:64, :], mapT_maxP[:, 32:64],
                             cB('Kcat2')[:, 448:896], start=False, stop=True)
            nc.vector.tensor_copy(out=R_sb[:, :], in_=ps_R[:, :])
            ps_sa = psC.tile([64, 64], F32, tag="sm2", name="ps_sa")
            for dy in range(7):
                nc.tensor.matmul(ps_sa[:, :], cB('identS')[:, dy:dy + 64],
                                 R_sb[:, bass.ts(dy, 64)],
                                 start=(dy == 0), stop=(dy == 6))
            nc.scalar.activation(out=sig2d[:, :], in_=ps_sa[:, :], func=AF.Sigmoid)

            # sigb broadcast straight from sig2d: stationary is a broadcast
            # w_fuse*ident column (selects row y), moving is the whole map
            # out = zs*sigb + xp, per-group pipelined with DMA out
            for t in range(8):
                ps_bc = psC.tile([128, 512], F32, tag="bc", name="ps_bc")
                for yl in range(8):
                    y = 8 * t + yl
                    nc.tensor.matmul(ps_bc[:, bass.ts(yl, 64)],
                                     cB('wident')[:, y:y + 1].broadcast_to(
                                         [64, 128]),
                                     sig2d[:, :], start=True, stop=True)
                nc.scalar.activation(out=sigb[:, 0, bass.ts(t, 512)],
                                     in_=ps_bc[:, :], func=AF.Copy)
                sl = bass.ts(t, 512)
                if t % 2 == 0:
                    vt2 = sp.tile([128, 2, 1024], BF16, tag="vt2",
                                  name="vt2", bufs=4)
                sgb = sigb[:, :, sl].broadcast_to([128, 2, 512])
                nc.vector.tensor_tensor(
                    out=vt2[:, :, 512 * (t % 2):512 * (t % 2) + 512],
                    in0=z_t[:, :, sl], in1=sgb, op=ALU.mult)
                if t % 2 == 1:
                    g = t // 2
                    nc.gpsimd.dma_start(
                        out=out_d[:, bass.ts(g, 1024)].rearrange(
                            "(two p) n -> p two n", two=2),
                        in_=vt2[:, :, :], accum_op=ALU.add)
        stack.close()
        for fr in reversed(_frees):
            fr()
    nc.compile()
    return nc


_CACHE = {}


def kernel(**inputs):
    inp = {k: np.asarray(v) for k, v in inputs.items()}
    f = fold_params(inp)
    key = round(f['w_fuse'], 9)
    if key not in _CACHE:
        _CACHE[key] = build_nc(f['w_fuse'])
    nc = _CACHE[key]

    B = inp['x'].shape[0]
    in_maps = []
    for b in range(B):
        xb = inp['x'][b].reshape(256, N).astype(np.float32)
        m = {
            'x': np.ascontiguousarray(
                xb.reshape(2, 128, N).transpose(1, 0, 2)).astype(ml_dtypes.bfloat16),
            'x0': np.ascontiguousarray(
                inp['x0'][b].reshape(128, N)).astype(ml_dtypes.bfloat16),
            'cbA': f['cbA'], 'cbB': f['cbB'], 'cf': f['cf'],
        }
        in_maps.append(m)

    res = run_bass_kernel_spmd(nc, in_maps, core_ids=list(range(B)))
    out = np.stack([np.asarray(res.results[b]['out'], dtype=np.float32
                               ).reshape(256, H, W) for b in range(B)])
    return out
